# revision 11
# baseline (speedup 1.0000x reference)
"""Bahdanau-style attention kernel for Trainium2, data-parallel over batch
across 8 NeuronCores.  v4: the host pre-packs the mask-selected encoder rows
(exp(-1e10+x) underflows to 0, so dropped rows are exact), pre-casts to bf16,
and uploads BOTH layouts -- enc [s-part, e] for the context matmul and
encT [e-part, s] for the projection matmul -- so the device does no gather,
no PE transposes and no PSUM->SBUF relayout copies.

Per batch b (reference):
    W_h, W_e = W_attn[:H], W_attn[H:]
    proj   = hidden @ W_h + enc[b] @ W_e + b_attn          # [S, H]
    energy = tanh(proj);  scores = energy @ W_v            # [S]
    attn   = softmax(where(mask==0, -1e10, scores))
    ctx    = attn @ enc[b]                                 # [2H]

Device dataflow (per core, 4 batches, s_p packed rows):
  projT[h,s] = sum_e W_e[e,h]^T encT[e,s]   PE, W_e chunks stationary (bf16)
  energyT    = tanh(projT + hb[h])          ACT, hb as per-partition bias
  scoresT    = sum_h wv[h] energyT[h,s]     PE, enT chunks stationary, N=1
  softmax over packed columns (pad bias -1e10), denom via ones-matmul
  ctxT[e]    = sum_s enc[s,e]^T attn[s]     PE, enc chunks stationary, N=1

Numerics: bf16 matmul datapath, f32 softmax.  rel err ~5e-3 (tol 2e-2).
"""

import math
import numpy as np
from ml_dtypes import bfloat16

B, S, H = 32, 1024, 512
E = 2 * H             # 1024
N_CORES = 8
B_LOC = B // N_CORES  # 4
HC = H // 128         # 4 output h-chunks
KE = E // 128         # 8 contraction e-chunks
KH = H // 128         # 4 contraction chunks for hidden @ W_h
NEG = -1e10

_cache = {}


def _install_tile_drain_patch():
    """walrus in this container rejects >1 sem-wait on the SP CTRL drain that
    TileContext emits at kernel tail; split the waits across 1-wait nops."""
    import concourse.tile as tile
    import concourse.mybir as mybir
    from concourse.vector_clock import ScopedClock

    if getattr(tile.TileContext, "_drain_patch_installed", False):
        return

    def _drain_and_barrier_split(self, tick_clock, wait_clock):
        nc = self.nc
        probe = nc.sync.nop(nofuse=True, hint="tail_wait_probe")
        wait_clock.add_sem_waits(
            probe.ins, ScopedClock({None: tick_clock.global_clock})
        )
        si = probe.ins.sync_info
        waits = list(si.on_wait) if si and si.on_wait else []
        if len(waits) > 1:
            si.on_wait = waits[:1]
            for w in waits[1:]:
                n = nc.sync.nop(nofuse=True, hint="tail_wait_extra")
                nsi = n.ins.sync_info
                if nsi is None:
                    n.ins.sync_info = mybir.SyncInfo(on_wait=[w], on_update=[])
                else:
                    nsi.on_wait = [w]
        nc.sync.drain()
        nc.all_engine_barrier()
        assert self.sems is not None
        popped = nc._tile_sem_poison_stack.pop()
        assert popped is self._sem_poison
        # chunked clear_and_free_semaphores: walrus rejects RANGE_CLEAR ISA
        # instructions spanning more than a few semaphores ("ISA wrong
        # length"), so clear in <=3-wide ranges.
        sems = list(self.sems.allocated().values())
        sem_nums = sorted(s.num if hasattr(s, "num") else s for s in sems)
        if sem_nums:
            runs = []
            lo = prev = sem_nums[0]
            for n in sem_nums[1:]:
                if n == prev + 1:
                    prev = n
                else:
                    runs.append((lo, prev))
                    lo = prev = n
            runs.append((lo, prev))
            for lo, hi in runs:
                for c0 in range(lo, hi + 1, 3):
                    c1 = min(c0 + 2, hi)
                    r = range(c0, c1 + 1)
                    assert nc._state.free_isdisjoint(r)
                    nc.gpsimd.dma_reset(r)
                    nc.gpsimd.sem_clear(r)
            nc._state.prepend_free_semaphores(sem_nums)
            for poison_set in nc._tile_sem_poison_stack:
                poison_set.update(sem_nums)
        nc.all_engine_barrier()

    tile.TileContext._drain_and_barrier = _drain_and_barrier_split
    tile.TileContext._drain_patch_installed = True


def _split_multiwaits(nc, max_waits=1):
    """walrus's setupSyncWait rejects instructions carrying more than a couple
    of semaphore waits.  Move excess waits onto same-engine nops inserted
    immediately before the offending instruction (engine executes in order, so
    semantics are identical)."""
    import concourse.mybir as mybir

    for f in nc.m.functions:
        for bb in f.blocks:
            out = []
            for inst in bb.instructions:
                si = inst.sync_info
                waits = list(si.on_wait) if si and si.on_wait else []
                lim = max_waits
                if len(waits) > lim:
                    excess = waits[:-lim]
                    si.on_wait = waits[-lim:]
                    for i in range(0, len(excess), max_waits):
                        nop = mybir.InstNoOp(
                            name=f"I-{nc.next_id()}-waitsplit", ins=[], outs=[]
                        )
                        nop.engine = inst.engine
                        nop.sync_info = mybir.SyncInfo(
                            on_wait=excess[i:i + max_waits], on_update=[]
                        )
                        nc.register_instruction(nop, overwrite=True)
                        out.append(nop)
                out.append(inst)
            bb.instructions[:] = out


def build_kernel(n_iters: int = 1, s_p: int = 560):
    """Per-core Bass program for packed row count s_p (multiple of 16).
    n_iters>1 repeats the compute body for slope-based timing."""
    _install_tile_drain_patch()
    import concourse.bass as bass
    import concourse.tile as tile
    import concourse.mybir as mybir
    from concourse.mybir import ActivationFunctionType as act

    f32 = mybir.dt.float32
    bf16 = mybir.dt.bfloat16

    ST = (s_p + 127) // 128          # s-tiles (last may be partial)
    R_LAST = s_p - 128 * (ST - 1)    # rows in last s-tile
    SA = min(512, s_p)               # PSUM s-split: [0,SA) + [SA,s_p)
    SB = s_p - SA
    STF = ST * 128                   # padded col count for col-layout tiles

    nc = bass.Bass("TRN2", target_bir_lowering=False, debug=False,
                   num_devices=N_CORES)

    encp_d = nc.dram_tensor("encp", [B_LOC, 128, ST * E], bf16,
                            kind="ExternalInput").ap()
    enctp_d = nc.dram_tensor("enctp", [B_LOC, 128, KE * s_p], bf16,
                             kind="ExternalInput").ap()
    wh_d = nc.dram_tensor("wh_sb", [128, KH * HC * 128], bf16,
                          kind="ExternalInput").ap()
    we_d = nc.dram_tensor("we_sb", [128, KE * HC * 128], bf16,
                          kind="ExternalInput").ap()
    hidT_d = nc.dram_tensor("hidT", [128, KH * B_LOC], bf16,
                            kind="ExternalInput").ap()
    wv_d = nc.dram_tensor("wv_col", [128, HC], bf16,
                          kind="ExternalInput").ap()
    battn_d = nc.dram_tensor("battn_row", [1, H], bf16,
                             kind="ExternalInput").ap()
    pb_d = nc.dram_tensor("pb_cols", [128, B_LOC * ST], f32,
                          kind="ExternalInput").ap()
    ctx_d = nc.dram_tensor("out_ctx", [B_LOC, E], f32,
                           kind="ExternalOutput").ap()
    attn_d = nc.dram_tensor("out_attn", [B_LOC, s_p], f32,
                            kind="ExternalOutput").ap()

    with tile.TileContext(nc) as tc:
        with (
            tc.tile_pool(name="const", bufs=1) as cpool,
            tc.tile_pool(name="enc", bufs=3) as encpool,
            tc.tile_pool(name="encT", bufs=3) as encTpool,
            tc.tile_pool(name="enrg", bufs=9) as enpool,
            tc.tile_pool(name="perb", bufs=3) as bpool,
            tc.tile_pool(name="psA", bufs=2, space="PSUM") as psA_pool,
            tc.tile_pool(name="psB", bufs=2, space="PSUM") as psB_pool,
            tc.tile_pool(name="pscol", bufs=2, space="PSUM") as pscol_pool,
            tc.tile_pool(name="pctxT", bufs=1, space="PSUM") as pctx_pool,
            tc.tile_pool(name="psm", bufs=1, space="PSUM") as psm_pool,
        ):
            # ---------------- constants / weights / small inputs ----------
            # (small tensors on the SP HWDGE ring; big ones on the ACT ring)
            hidT = cpool.tile([128, KH * B_LOC], bf16)
            nc.sync.dma_start(hidT[:], hidT_d)
            wv_col = cpool.tile([128, HC], bf16)
            nc.sync.dma_start(wv_col[:], wv_d)
            battn_row = cpool.tile([1, H], bf16)
            nc.sync.dma_start(battn_row[:], battn_d)
            pb_sb = cpool.tile([128, B_LOC * ST], f32)
            nc.sync.dma_start(pb_sb[:], pb_d)
            w_h = cpool.tile([128, KH * HC * 128], bf16)
            nc.sync.dma_start(w_h[:], wh_d)

            w_e = cpool.tile([128, KE * HC * 128], bf16)
            nc.scalar.dma_start(w_e[:], we_d)
            encT_first = encTpool.tile([128, KE * s_p], bf16, tag="encT")
            nc.scalar.dma_start(encT_first[:], enctp_d[0])
            enc_first = encpool.tile([128, ST * E], bf16, tag="enc")
            nc.sync.dma_start(enc_first[:], encp_d[0])

            ones4 = cpool.tile([1, B_LOC], bf16)
            nc.vector.memset(ones4[:], 1.0)
            ones_col = cpool.tile([128, 1], f32)
            nc.vector.memset(ones_col[:], 1.0)
            ones_row = cpool.tile([1, 128], f32)
            nc.vector.memset(ones_row[:], 1.0)

            # hbT[:, hc*B_LOC + b] = (hidden @ W_h + b_attn)[b, hc*128:+128]
            hbT = cpool.tile([128, HC * B_LOC], f32)

            def emit_preamble():
                """PE matmuls for the per-batch tanh bias; depends only on
                the small DMAs so it runs while the big enc loads land."""
                for hc in range(HC):
                    p_ph = psm_pool.tile([128, B_LOC], f32, tag="sm")
                    for k in range(KH):
                        nc.tensor.matmul(
                            p_ph[:],
                            w_h[:, (k * HC + hc) * 128:(k * HC + hc + 1) * 128],
                            hidT[:, k * B_LOC:(k + 1) * B_LOC],
                            start=(k == 0), stop=False,
                        )
                    nc.tensor.matmul(
                        p_ph[:], battn_row[:, hc * 128:(hc + 1) * 128],
                        ones4[:], start=False, stop=True,
                    )
                    nc.any.tensor_copy(hbT[:, hc * B_LOC:(hc + 1) * B_LOC],
                                       p_ph[:])

            # ---------------- deferred emission machinery -----------------
            def emit_scores(sc):
                """Scores for a finished batch.  PSUM accumulation groups must
                be contiguous per bank (start=True wipes the whole bank), so
                each column's HC members are emitted back-to-back."""
                b, enTs, p_scol = sc
                for st in range(ST):
                    for hc in range(HC):
                        nc.tensor.matmul(
                            p_scol[:, st:st + 1],
                            enTs[hc][:, st * 128:(st + 1) * 128],
                            wv_col[:, hc:hc + 1],
                            start=(hc == 0), stop=(hc == HC - 1),
                        )

            def emit_tail(pend):
                """Softmax + context for a finished batch."""
                pb, enc_b, p_scol = pend
                sm = bpool.tile([128, ST], f32, tag="sm")
                nc.vector.tensor_add(
                    sm[:], p_scol[:], pb_sb[:, pb * ST:(pb + 1) * ST]
                )
                p_exp = bpool.tile([128, ST], f32, tag="p_exp")
                rowsum = bpool.tile([128, 1], f32, tag="rowsum")
                nc.scalar.activation(p_exp[:], sm[:], act.Exp,
                                     accum_out=rowsum[:])
                p_den = psm_pool.tile([1, 1], f32, tag="sm")
                nc.tensor.matmul(p_den[:], rowsum[:], ones_col[:],
                                 start=True, stop=True)
                rd = bpool.tile([1, 1], f32, tag="rd")
                nc.vector.reciprocal(rd[:], p_den[:])
                p_rb = psm_pool.tile([128, 1], f32, tag="sm")
                nc.tensor.matmul(p_rb[:], ones_row[:], rd[:],
                                 start=True, stop=True)
                rb = bpool.tile([128, 1], f32, tag="rb")
                nc.any.tensor_copy(rb[:], p_rb[:])
                attn_sb = bpool.tile([128, ST], f32, tag="attn_sb")
                nc.vector.tensor_scalar_mul(attn_sb[:], p_exp[:], rb[:])
                p_r = bpool.tile([128, ST], bf16, tag="p_r")
                nc.vector.tensor_scalar_mul(p_r[:], p_exp[:], rb[:])
                # attention output (packed)
                if ST > 1:
                    nc.sync.dma_start(
                        attn_d[pb][0:128 * (ST - 1)].rearrange(
                            "(st p) -> p st", p=128),
                        attn_sb[:, 0:ST - 1],
                    )
                nc.sync.dma_start(
                    attn_d[pb][128 * (ST - 1):s_p].rearrange(
                        "(st p) -> p st", p=R_LAST),
                    attn_sb[0:R_LAST, ST - 1:ST],
                )
                # ctxT[e] = sum_s enc[s, e] * attn[s]: enc chunks stationary
                p_ctxT = pctx_pool.tile([128, KE], f32, tag="p_ctxT")
                for ec in range(KE):
                    for st in range(ST):
                        rows = 128 if st < ST - 1 else R_LAST
                        nc.tensor.matmul(
                            p_ctxT[:, ec:ec + 1],
                            enc_b[0:rows,
                                  st * E + ec * 128:st * E + (ec + 1) * 128],
                            p_r[0:rows, st:st + 1],
                            start=(st == 0), stop=(st == ST - 1),
                        )
                ctx_sb = bpool.tile([128, KE], f32, tag="ctx_sb")
                nc.any.tensor_copy(ctx_sb[:], p_ctxT[:])
                nc.sync.dma_start(
                    ctx_d[pb].rearrange("(c p) -> p c", p=128), ctx_sb[:]
                )

            # ---------------- main loop -----------------------------------
            emit_preamble()
            pend_sc = None    # scores group awaiting emission
            pend_tail = None  # finished batch awaiting softmax+ctx
            enc_b = enc_first
            encT_b = encT_first
            enc_next = encT_next = None
            for it in range(n_iters):
                for b in range(B_LOC):
                    if not (it == 0 and b == 0):
                        enc_b, encT_b = enc_next, encT_next
                    last_batch = (it == n_iters - 1 and b == B_LOC - 1)
                    if not last_batch:
                        nb = (b + 1) % B_LOC
                        encT_next = encTpool.tile([128, KE * s_p], bf16,
                                                  tag="encT")
                        nc.scalar.dma_start(encT_next[:], enctp_d[nb])
                        enc_next = encpool.tile([128, ST * E], bf16,
                                                tag="enc")
                        nc.sync.dma_start(enc_next[:], encp_d[nb])

                    p_scol = pscol_pool.tile([128, ST], f32, tag="p_scol")
                    enTs = []
                    for hc in range(HC):
                        pA = psA_pool.tile([128, SA], f32, tag="pA")
                        if SB:
                            pB = psB_pool.tile([128, SB], f32, tag="pB")
                        for k in range(KE):
                            lhs = w_e[:, (k * HC + hc) * 128:
                                      (k * HC + hc + 1) * 128]
                            nc.tensor.matmul(
                                pA[:], lhs,
                                encT_b[:, k * s_p:k * s_p + SA],
                                start=(k == 0), stop=(k == KE - 1),
                            )
                            if SB:
                                nc.tensor.matmul(
                                    pB[:], lhs,
                                    encT_b[:, k * s_p + SA:(k + 1) * s_p],
                                    start=(k == 0), stop=(k == KE - 1),
                                )
                        if hc == 1 and pend_sc is not None:
                            emit_scores(pend_sc)
                            pend_sc = None
                        if hc == 2 and pend_tail is not None:
                            emit_tail(pend_tail)
                            pend_tail = None
                        # energyT = tanh(projT + hb), padded cols zeroed so
                        # the full-width scores matmul stays NaN-free
                        enT = enpool.tile([128, STF], bf16, tag="enT")
                        if STF > s_p:
                            nc.vector.memset(enT[:, s_p:STF], 0.0)
                        hb_col = hbT[:, hc * B_LOC + b:hc * B_LOC + b + 1]
                        nc.scalar.activation(enT[:, 0:SA], pA[:], act.Tanh,
                                             bias=hb_col)
                        if SB:
                            nc.scalar.activation(enT[:, SA:s_p], pB[:],
                                                 act.Tanh, bias=hb_col)
                        enTs.append(enT)
                    pend_sc = (b, enTs, p_scol)
                    pend_tail_next = (b, enc_b, p_scol)
                    if pend_tail is not None:
                        emit_tail(pend_tail)
                    pend_tail = pend_tail_next
                # end b loop
            if pend_sc is not None:
                emit_scores(pend_sc)
                pend_sc = None
            if pend_tail is not None:
                emit_tail(pend_tail)
                pend_tail = None

    _split_multiwaits(nc)
    import concourse.mybir as mybir2
    mybir2.codegen_inst_isa_subclasses(nc)
    return nc


def _get_nc(n_iters: int = 1, s_p: int = 560):
    key = ("nc", n_iters, s_p)
    if key not in _cache:
        _cache[key] = build_kernel(n_iters, s_p)
    return _cache[key]


def pick_s_p(mask):
    counts = mask.reshape(B, S).sum(1)
    m = int(counts.max())
    return max(128, min(S, ((m + 15) // 16) * 16))


def _pack_core(enc_c, mask_c, s_p):
    """Host-side pack: mask-selected rows in tile layout (both orientations,
    bf16) + pad bias columns + scatter info."""
    ST = (s_p + 127) // 128
    encp = np.zeros((B_LOC, 128, ST * E), bfloat16)
    enctp = np.zeros((B_LOC, 128, KE * s_p), bfloat16)
    pb_pad = np.full((B_LOC, ST * 128), NEG, np.float32)
    scat = []
    for b in range(B_LOC):
        idx = np.nonzero(mask_c[b])[0]
        n = len(idx)
        rows = enc_c[b, idx].astype(bfloat16)          # [n, E]
        buf = np.zeros((ST * 128, E), bfloat16)
        buf[:n] = rows
        encp[b] = buf.reshape(ST, 128, E).transpose(1, 0, 2).reshape(
            128, ST * E)
        t = np.zeros((s_p, E), bfloat16)
        t[:n] = rows
        enctp[b] = t.T.reshape(KE, 128, s_p).transpose(1, 0, 2).reshape(
            128, KE * s_p)
        pb_pad[b, :n] = 0.0
        scat.append((n, idx.astype(np.int64)))
    pb_cols = pb_pad.reshape(B_LOC, ST, 128).transpose(2, 0, 1).reshape(
        128, B_LOC * ST)
    return encp, enctp, np.ascontiguousarray(pb_cols), scat


def shard_inputs(hidden, encoder_outputs, mask, W_attn, b_attn, W_v,
                 s_p=None):
    hidden = np.asarray(hidden, dtype=np.float32)
    enc = np.asarray(encoder_outputs, dtype=np.float32)
    mask = np.asarray(mask, dtype=np.int32)
    W_attn = np.asarray(W_attn, dtype=np.float32)
    b_attn = np.asarray(b_attn, dtype=np.float32)
    W_v = np.asarray(W_v, dtype=np.float32)
    if s_p is None:
        s_p = pick_s_p(mask)

    w_h = W_attn[:H].astype(bfloat16)                  # [512, 512]
    w_e = W_attn[H:].astype(bfloat16)                  # [1024, 512]
    wh_sb = np.ascontiguousarray(
        w_h.reshape(KH, 128, HC, 128).transpose(1, 0, 2, 3).reshape(
            128, KH * HC * 128))
    we_sb = np.ascontiguousarray(
        w_e.reshape(KE, 128, HC, 128).transpose(1, 0, 2, 3).reshape(
            128, KE * HC * 128))
    wv_col = np.ascontiguousarray(
        W_v.astype(bfloat16).reshape(HC, 128).T)       # [128, HC]
    battn_row = np.ascontiguousarray(
        b_attn.astype(bfloat16)[None, :])              # [1, H]

    in_maps, scats = [], []
    for c in range(N_CORES):
        sl = slice(c * B_LOC, (c + 1) * B_LOC)
        hidT = np.ascontiguousarray(
            hidden[sl].astype(bfloat16).T.reshape(
                KH, 128, B_LOC).transpose(1, 0, 2).reshape(128, KH * B_LOC))
        encp, enctp, pb_cols, scat = _pack_core(enc[sl], mask[sl], s_p)
        in_maps.append({
            "encp": encp,
            "enctp": enctp,
            "wh_sb": wh_sb,
            "we_sb": we_sb,
            "hidT": hidT,
            "wv_col": wv_col,
            "battn_row": battn_row,
            "pb_cols": pb_cols,
        })
        scats.append(scat)
    return in_maps, scats, s_p


def kernel(hidden, encoder_outputs, mask, W_attn, b_attn, W_v):
    from concourse.bass_utils import run_bass_kernel_spmd

    in_maps, scats, s_p = shard_inputs(
        hidden, encoder_outputs, mask, W_attn, b_attn, W_v)
    nc = _get_nc(1, s_p)
    res = run_bass_kernel_spmd(nc, in_maps, list(range(N_CORES)))
    context = np.concatenate(
        [res.results[c]["out_ctx"] for c in range(N_CORES)], 0)
    attn_p = np.concatenate(
        [res.results[c]["out_attn"] for c in range(N_CORES)], 0)
    attn_w = np.zeros((B, S), np.float32)
    for c in range(N_CORES):
        for b in range(B_LOC):
            n, idx = scats[c][b]
            attn_w[c * B_LOC + b, idx] = attn_p[c * B_LOC + b, :n]
    return context.astype(np.float32), attn_w.astype(np.float32)


# revision 12
# speedup vs baseline: 2.3432x; 2.3432x over previous
"""Bahdanau-style attention kernel for Trainium2, data-parallel over batch
across 8 NeuronCores.  v4: the host pre-packs the mask-selected encoder rows
(exp(-1e10+x) underflows to 0, so dropped rows are exact), pre-casts to bf16,
and uploads BOTH layouts -- enc [s-part, e] for the context matmul and
encT [e-part, s] for the projection matmul -- so the device does no gather,
no PE transposes and no PSUM->SBUF relayout copies.

Per batch b (reference):
    W_h, W_e = W_attn[:H], W_attn[H:]
    proj   = hidden @ W_h + enc[b] @ W_e + b_attn          # [S, H]
    energy = tanh(proj);  scores = energy @ W_v            # [S]
    attn   = softmax(where(mask==0, -1e10, scores))
    ctx    = attn @ enc[b]                                 # [2H]

Device dataflow (per core, 4 batches, s_p packed rows):
  projT[h,s] = sum_e W_e[e,h]^T encT[e,s]   PE, W_e chunks stationary (bf16)
  energyT    = tanh(projT + hb[h])          ACT, hb as per-partition bias
  scoresT    = sum_h wv[h] energyT[h,s]     PE, enT chunks stationary, N=1
  softmax over packed columns (pad bias -1e10), denom via ones-matmul
  ctxT[e]    = sum_s enc[s,e]^T attn[s]     PE, enc chunks stationary, N=1

Numerics: bf16 matmul datapath, f32 softmax.  rel err ~5e-3 (tol 2e-2).
"""

import math
import numpy as np
from ml_dtypes import bfloat16

B, S, H = 32, 1024, 512
E = 2 * H             # 1024
N_CORES = 8
B_LOC = B // N_CORES  # 4
HC = H // 128         # 4 output h-chunks
KE = E // 128         # 8 contraction e-chunks
KH = H // 128         # 4 contraction chunks for hidden @ W_h
NEG = -1e10

_cache = {}


def _install_tile_drain_patch():
    """walrus in this container rejects >1 sem-wait on the SP CTRL drain that
    TileContext emits at kernel tail; split the waits across 1-wait nops."""
    import concourse.tile as tile
    import concourse.mybir as mybir
    from concourse.vector_clock import ScopedClock

    if getattr(tile.TileContext, "_drain_patch_installed", False):
        return

    def _drain_and_barrier_split(self, tick_clock, wait_clock):
        nc = self.nc
        probe = nc.sync.nop(nofuse=True, hint="tail_wait_probe")
        wait_clock.add_sem_waits(
            probe.ins, ScopedClock({None: tick_clock.global_clock})
        )
        si = probe.ins.sync_info
        waits = list(si.on_wait) if si and si.on_wait else []
        if len(waits) > 1:
            si.on_wait = waits[:1]
            for w in waits[1:]:
                n = nc.sync.nop(nofuse=True, hint="tail_wait_extra")
                nsi = n.ins.sync_info
                if nsi is None:
                    n.ins.sync_info = mybir.SyncInfo(on_wait=[w], on_update=[])
                else:
                    nsi.on_wait = [w]
        nc.sync.drain()
        nc.all_engine_barrier()
        assert self.sems is not None
        popped = nc._tile_sem_poison_stack.pop()
        assert popped is self._sem_poison
        # chunked clear_and_free_semaphores: walrus rejects RANGE_CLEAR ISA
        # instructions spanning more than a few semaphores ("ISA wrong
        # length"), so clear in <=3-wide ranges.
        sems = list(self.sems.allocated().values())
        sem_nums = sorted(s.num if hasattr(s, "num") else s for s in sems)
        if sem_nums:
            runs = []
            lo = prev = sem_nums[0]
            for n in sem_nums[1:]:
                if n == prev + 1:
                    prev = n
                else:
                    runs.append((lo, prev))
                    lo = prev = n
            runs.append((lo, prev))
            for lo, hi in runs:
                for c0 in range(lo, hi + 1, 3):
                    c1 = min(c0 + 2, hi)
                    r = range(c0, c1 + 1)
                    assert nc._state.free_isdisjoint(r)
                    nc.gpsimd.dma_reset(r)
                    nc.gpsimd.sem_clear(r)
            nc._state.prepend_free_semaphores(sem_nums)
            for poison_set in nc._tile_sem_poison_stack:
                poison_set.update(sem_nums)
        nc.all_engine_barrier()

    tile.TileContext._drain_and_barrier = _drain_and_barrier_split
    tile.TileContext._drain_patch_installed = True


def _split_multiwaits(nc, max_waits=1):
    """walrus's setupSyncWait rejects instructions carrying more than a couple
    of semaphore waits.  Move excess waits onto same-engine nops inserted
    immediately before the offending instruction (engine executes in order, so
    semantics are identical)."""
    import concourse.mybir as mybir

    for f in nc.m.functions:
        for bb in f.blocks:
            out = []
            for inst in bb.instructions:
                si = inst.sync_info
                waits = list(si.on_wait) if si and si.on_wait else []
                lim = max_waits
                if len(waits) > lim:
                    excess = waits[:-lim]
                    si.on_wait = waits[-lim:]
                    for i in range(0, len(excess), max_waits):
                        nop = mybir.InstNoOp(
                            name=f"I-{nc.next_id()}-waitsplit", ins=[], outs=[]
                        )
                        nop.engine = inst.engine
                        nop.sync_info = mybir.SyncInfo(
                            on_wait=excess[i:i + max_waits], on_update=[]
                        )
                        nc.register_instruction(nop, overwrite=True)
                        out.append(nop)
                out.append(inst)
            bb.instructions[:] = out


def build_kernel(n_iters: int = 1, s_p: int = 560):
    """Per-core Bass program for packed row count s_p (multiple of 16).
    n_iters>1 repeats the compute body for slope-based timing."""
    _install_tile_drain_patch()
    import concourse.bass as bass
    import concourse.tile as tile
    import concourse.mybir as mybir
    from concourse.mybir import ActivationFunctionType as act

    f32 = mybir.dt.float32
    bf16 = mybir.dt.bfloat16

    ST = (s_p + 127) // 128          # s-tiles (last may be partial)
    R_LAST = s_p - 128 * (ST - 1)    # rows in last s-tile
    SA = min(512, s_p)               # PSUM s-split: [0,SA) + [SA,s_p)
    SB = s_p - SA
    STF = ST * 128                   # padded col count for col-layout tiles

    nc = bass.Bass("TRN2", target_bir_lowering=False, debug=False,
                   num_devices=N_CORES)

    encp_d = nc.dram_tensor("encp", [B_LOC, 128, ST * E], bf16,
                            kind="ExternalInput").ap()
    enctp_d = nc.dram_tensor("enctp", [B_LOC, 128, KE * s_p], bf16,
                             kind="ExternalInput").ap()
    wh_d = nc.dram_tensor("wh_sb", [128, KH * HC * 128], bf16,
                          kind="ExternalInput").ap()
    we_d = nc.dram_tensor("we_sb", [128, KE * HC * 128], bf16,
                          kind="ExternalInput").ap()
    hidT_d = nc.dram_tensor("hidT", [128, KH * B_LOC], bf16,
                            kind="ExternalInput").ap()
    wv_d = nc.dram_tensor("wv_col", [128, HC], bf16,
                          kind="ExternalInput").ap()
    battn_d = nc.dram_tensor("battn_row", [1, H], bf16,
                             kind="ExternalInput").ap()
    pb_d = nc.dram_tensor("pb_cols", [128, B_LOC * ST], f32,
                          kind="ExternalInput").ap()
    # column-layout outputs (one contiguous [128, ST+KE] block per batch;
    # host unpacks): out[:, 0:ST] = attn cols, out[:, ST:ST+KE] = ctxT
    out_d = nc.dram_tensor("out_cols", [B_LOC, 128, ST + KE], f32,
                           kind="ExternalOutput").ap()

    with tile.TileContext(nc) as tc:
        with (
            tc.tile_pool(name="const", bufs=1) as cpool,
            tc.tile_pool(name="enc", bufs=3) as encpool,
            tc.tile_pool(name="encT", bufs=3) as encTpool,
            tc.tile_pool(name="enrg", bufs=9) as enpool,
            tc.tile_pool(name="perb", bufs=3) as bpool,
            tc.tile_pool(name="psA", bufs=2, space="PSUM") as psA_pool,
            tc.tile_pool(name="psB", bufs=2, space="PSUM") as psB_pool,
            tc.tile_pool(name="pscol", bufs=2, space="PSUM") as pscol_pool,
            tc.tile_pool(name="pctxT", bufs=1, space="PSUM") as pctx_pool,
            tc.tile_pool(name="psm", bufs=1, space="PSUM") as psm_pool,
        ):
            # ---------------- constants / weights / small inputs ----------
            # (small tensors on the SP HWDGE ring; big ones on the ACT ring)
            hidT = cpool.tile([128, KH * B_LOC], bf16)
            nc.sync.dma_start(hidT[:], hidT_d)
            wv_col = cpool.tile([128, HC], bf16)
            nc.sync.dma_start(wv_col[:], wv_d)
            battn_row = cpool.tile([1, H], bf16)
            nc.sync.dma_start(battn_row[:], battn_d)
            pb_sb = cpool.tile([128, B_LOC * ST], f32)
            nc.sync.dma_start(pb_sb[:], pb_d)
            w_h = cpool.tile([128, KH * HC * 128], bf16)
            nc.sync.dma_start(w_h[:], wh_d)

            w_e = cpool.tile([128, KE * HC * 128], bf16)
            nc.scalar.dma_start(w_e[:], we_d)
            encT_first = encTpool.tile([128, KE * s_p], bf16, tag="encT")
            nc.scalar.dma_start(encT_first[:], enctp_d[0])
            enc_first = encpool.tile([128, ST * E], bf16, tag="enc")
            nc.sync.dma_start(enc_first[:, 0:(ST - 1) * E],
                              encp_d[0][:, 0:(ST - 1) * E])
            nc.sync.dma_start(enc_first[0:R_LAST, (ST - 1) * E:ST * E],
                              encp_d[0][0:R_LAST, (ST - 1) * E:ST * E])

            ones4 = cpool.tile([1, B_LOC], bf16)
            nc.vector.memset(ones4[:], 1.0)
            ones_col = cpool.tile([128, 1], f32)
            nc.vector.memset(ones_col[:], 1.0)
            ones_row = cpool.tile([1, 128], f32)
            nc.vector.memset(ones_row[:], 1.0)

            # hbT[:, hc*B_LOC + b] = (hidden @ W_h + b_attn)[b, hc*128:+128]
            hbT = cpool.tile([128, HC * B_LOC], f32)

            def emit_preamble():
                """PE matmuls for the per-batch tanh bias; depends only on
                the small DMAs so it runs while the big enc loads land."""
                for hc in range(HC):
                    p_ph = psm_pool.tile([128, B_LOC], f32, tag="sm")
                    for k in range(KH):
                        nc.tensor.matmul(
                            p_ph[:],
                            w_h[:, (k * HC + hc) * 128:(k * HC + hc + 1) * 128],
                            hidT[:, k * B_LOC:(k + 1) * B_LOC],
                            start=(k == 0), stop=False,
                        )
                    nc.tensor.matmul(
                        p_ph[:], battn_row[:, hc * 128:(hc + 1) * 128],
                        ones4[:], start=False, stop=True,
                    )
                    nc.any.tensor_copy(hbT[:, hc * B_LOC:(hc + 1) * B_LOC],
                                       p_ph[:])

            # ---------------- deferred emission machinery -----------------
            def emit_scores(sc):
                """Scores for a finished batch.  PSUM accumulation groups must
                be contiguous per bank (start=True wipes the whole bank), so
                each column's HC members are emitted back-to-back."""
                b, enTs, p_scol = sc
                for st in range(ST):
                    for hc in range(HC):
                        nc.tensor.matmul(
                            p_scol[:, st:st + 1],
                            enTs[hc][:, st * 128:(st + 1) * 128],
                            wv_col[:, hc:hc + 1],
                            start=(hc == 0), stop=(hc == HC - 1),
                        )

            def emit_tail(pend):
                """Softmax + context for a finished batch."""
                pb, enc_b, p_scol = pend
                sm = bpool.tile([128, ST], f32, tag="sm")
                nc.vector.tensor_add(
                    sm[:], p_scol[:], pb_sb[:, pb * ST:(pb + 1) * ST]
                )
                p_exp = bpool.tile([128, ST], f32, tag="p_exp")
                rowsum = bpool.tile([128, 1], f32, tag="rowsum")
                nc.scalar.activation(p_exp[:], sm[:], act.Exp,
                                     accum_out=rowsum[:])
                p_den = psm_pool.tile([1, 1], f32, tag="sm")
                nc.tensor.matmul(p_den[:], rowsum[:], ones_col[:],
                                 start=True, stop=True)
                rd = bpool.tile([1, 1], f32, tag="rd")
                nc.vector.reciprocal(rd[:], p_den[:])
                p_rb = psm_pool.tile([128, 1], f32, tag="sm")
                nc.tensor.matmul(p_rb[:], ones_row[:], rd[:],
                                 start=True, stop=True)
                rb = bpool.tile([128, 1], f32, tag="rb")
                nc.any.tensor_copy(rb[:], p_rb[:])
                out_sb = bpool.tile([128, ST + KE], f32, tag="out_sb")
                nc.vector.tensor_scalar_mul(out_sb[:, 0:ST], p_exp[:], rb[:])
                p_r = bpool.tile([128, ST], bf16, tag="p_r")
                nc.vector.tensor_scalar_mul(p_r[:], p_exp[:], rb[:])
                # ctxT[e] = sum_s enc[s, e] * attn[s]: enc chunks stationary
                p_ctxT = pctx_pool.tile([128, KE], f32, tag="p_ctxT")
                for ec in range(KE):
                    for st in range(ST):
                        rows = 128 if st < ST - 1 else R_LAST
                        nc.tensor.matmul(
                            p_ctxT[:, ec:ec + 1],
                            enc_b[0:rows,
                                  st * E + ec * 128:st * E + (ec + 1) * 128],
                            p_r[0:rows, st:st + 1],
                            start=(st == 0), stop=(st == ST - 1),
                        )
                nc.any.tensor_copy(out_sb[:, ST:ST + KE], p_ctxT[:])
                nc.gpsimd.dma_start(out_d[pb], out_sb[:])

            # ---------------- main loop -----------------------------------
            emit_preamble()
            pend_sc = None    # scores group awaiting emission
            pend_tail = None  # finished batch awaiting softmax+ctx
            enc_b = enc_first
            encT_b = encT_first
            enc_next = encT_next = None
            for it in range(n_iters):
                for b in range(B_LOC):
                    if not (it == 0 and b == 0):
                        enc_b, encT_b = enc_next, encT_next
                    last_batch = (it == n_iters - 1 and b == B_LOC - 1)
                    if not last_batch:
                        nb = (b + 1) % B_LOC
                        encT_next = encTpool.tile([128, KE * s_p], bf16,
                                                  tag="encT")
                        nc.scalar.dma_start(encT_next[:], enctp_d[nb])
                        enc_next = encpool.tile([128, ST * E], bf16,
                                                tag="enc")
                        nc.sync.dma_start(enc_next[:, 0:(ST - 1) * E],
                                          encp_d[nb][:, 0:(ST - 1) * E])
                        nc.sync.dma_start(
                            enc_next[0:R_LAST, (ST - 1) * E:ST * E],
                            encp_d[nb][0:R_LAST, (ST - 1) * E:ST * E])

                    p_scol = pscol_pool.tile([128, ST], f32, tag="p_scol")
                    enTs = []
                    for hc in range(HC):
                        pA = psA_pool.tile([128, SA], f32, tag="pA")
                        if SB:
                            pB = psB_pool.tile([128, SB], f32, tag="pB")
                        for k in range(KE):
                            lhs = w_e[:, (k * HC + hc) * 128:
                                      (k * HC + hc + 1) * 128]
                            nc.tensor.matmul(
                                pA[:], lhs,
                                encT_b[:, k * s_p:k * s_p + SA],
                                start=(k == 0), stop=(k == KE - 1),
                            )
                            if SB:
                                nc.tensor.matmul(
                                    pB[:], lhs,
                                    encT_b[:, k * s_p + SA:(k + 1) * s_p],
                                    start=(k == 0), stop=(k == KE - 1),
                                )
                        if hc == 1 and pend_sc is not None:
                            emit_scores(pend_sc)
                            pend_sc = None
                        if hc == 2 and pend_tail is not None:
                            emit_tail(pend_tail)
                            pend_tail = None
                        # energyT = tanh(projT + hb), padded cols zeroed so
                        # the full-width scores matmul stays NaN-free
                        enT = enpool.tile([128, STF], bf16, tag="enT")
                        if STF > s_p:
                            nc.vector.memset(enT[:, s_p:STF], 0.0)
                        hb_col = hbT[:, hc * B_LOC + b:hc * B_LOC + b + 1]
                        nc.scalar.activation(enT[:, 0:SA], pA[:], act.Tanh,
                                             bias=hb_col)
                        if SB:
                            nc.scalar.activation(enT[:, SA:s_p], pB[:],
                                                 act.Tanh, bias=hb_col)
                        enTs.append(enT)
                    pend_sc = (b, enTs, p_scol)
                    pend_tail_next = (b, enc_b, p_scol)
                    if pend_tail is not None:
                        emit_tail(pend_tail)
                    pend_tail = pend_tail_next
                # end b loop
            if pend_sc is not None:
                emit_scores(pend_sc)
                pend_sc = None
            if pend_tail is not None:
                emit_tail(pend_tail)
                pend_tail = None

    _split_multiwaits(nc)
    import concourse.mybir as mybir2
    mybir2.codegen_inst_isa_subclasses(nc)
    return nc


def _get_nc(n_iters: int = 1, s_p: int = 560):
    key = ("nc", n_iters, s_p)
    if key not in _cache:
        _cache[key] = build_kernel(n_iters, s_p)
    return _cache[key]


def pick_s_p(mask):
    counts = mask.reshape(B, S).sum(1)
    m = int(counts.max())
    return max(128, min(S, ((m + 15) // 16) * 16))


def _pack_core(enc_c, mask_c, s_p):
    """Host-side pack: mask-selected rows in tile layout (both orientations,
    bf16) + pad bias columns + scatter info."""
    ST = (s_p + 127) // 128
    encp = np.zeros((B_LOC, 128, ST * E), bfloat16)
    enctp = np.zeros((B_LOC, 128, KE * s_p), bfloat16)
    pb_pad = np.full((B_LOC, ST * 128), NEG, np.float32)
    scat = []
    for b in range(B_LOC):
        idx = np.nonzero(mask_c[b])[0]
        n = len(idx)
        rows = enc_c[b, idx].astype(bfloat16)          # [n, E]
        buf = np.zeros((ST * 128, E), bfloat16)
        buf[:n] = rows
        encp[b] = buf.reshape(ST, 128, E).transpose(1, 0, 2).reshape(
            128, ST * E)
        t = np.zeros((s_p, E), bfloat16)
        t[:n] = rows
        enctp[b] = t.T.reshape(KE, 128, s_p).transpose(1, 0, 2).reshape(
            128, KE * s_p)
        pb_pad[b, :n] = 0.0
        scat.append((n, idx.astype(np.int64)))
    pb_cols = pb_pad.reshape(B_LOC, ST, 128).transpose(2, 0, 1).reshape(
        128, B_LOC * ST)
    return encp, enctp, np.ascontiguousarray(pb_cols), scat


def shard_inputs(hidden, encoder_outputs, mask, W_attn, b_attn, W_v,
                 s_p=None):
    hidden = np.asarray(hidden, dtype=np.float32)
    enc = np.asarray(encoder_outputs, dtype=np.float32)
    mask = np.asarray(mask, dtype=np.int32)
    W_attn = np.asarray(W_attn, dtype=np.float32)
    b_attn = np.asarray(b_attn, dtype=np.float32)
    W_v = np.asarray(W_v, dtype=np.float32)
    if s_p is None:
        s_p = pick_s_p(mask)

    w_h = W_attn[:H].astype(bfloat16)                  # [512, 512]
    w_e = W_attn[H:].astype(bfloat16)                  # [1024, 512]
    wh_sb = np.ascontiguousarray(
        w_h.reshape(KH, 128, HC, 128).transpose(1, 0, 2, 3).reshape(
            128, KH * HC * 128))
    we_sb = np.ascontiguousarray(
        w_e.reshape(KE, 128, HC, 128).transpose(1, 0, 2, 3).reshape(
            128, KE * HC * 128))
    wv_col = np.ascontiguousarray(
        W_v.astype(bfloat16).reshape(HC, 128).T)       # [128, HC]
    battn_row = np.ascontiguousarray(
        b_attn.astype(bfloat16)[None, :])              # [1, H]

    in_maps, scats = [], []
    for c in range(N_CORES):
        sl = slice(c * B_LOC, (c + 1) * B_LOC)
        hidT = np.ascontiguousarray(
            hidden[sl].astype(bfloat16).T.reshape(
                KH, 128, B_LOC).transpose(1, 0, 2).reshape(128, KH * B_LOC))
        encp, enctp, pb_cols, scat = _pack_core(enc[sl], mask[sl], s_p)
        in_maps.append({
            "encp": encp,
            "enctp": enctp,
            "wh_sb": wh_sb,
            "we_sb": we_sb,
            "hidT": hidT,
            "wv_col": wv_col,
            "battn_row": battn_row,
            "pb_cols": pb_cols,
        })
        scats.append(scat)
    return in_maps, scats, s_p


def kernel(hidden, encoder_outputs, mask, W_attn, b_attn, W_v):
    from concourse.bass_utils import run_bass_kernel_spmd

    in_maps, scats, s_p = shard_inputs(
        hidden, encoder_outputs, mask, W_attn, b_attn, W_v)
    nc = _get_nc(1, s_p)
    res = run_bass_kernel_spmd(nc, in_maps, list(range(N_CORES)))
    ST = (s_p + 127) // 128
    context = np.zeros((B, E), np.float32)
    attn_w = np.zeros((B, S), np.float32)
    for c in range(N_CORES):
        out = res.results[c]["out_cols"]          # [B_LOC, 128, ST+KE]
        for b in range(B_LOC):
            n, idx = scats[c][b]
            attn_p = out[b, :, 0:ST].T.reshape(-1)[:n]   # s = st*128 + p
            attn_w[c * B_LOC + b, idx] = attn_p
            context[c * B_LOC + b] = out[b, :, ST:].T.reshape(-1)  # e = ec*128+p
    return context, attn_w


# revision 13
# speedup vs baseline: 3.8259x; 1.6327x over previous
"""Bahdanau-style attention kernel for Trainium2, data-parallel over batch
across 8 NeuronCores.  v4: the host pre-packs the mask-selected encoder rows
(exp(-1e10+x) underflows to 0, so dropped rows are exact), pre-casts to bf16,
and uploads BOTH layouts -- enc [s-part, e] for the context matmul and
encT [e-part, s] for the projection matmul -- so the device does no gather,
no PE transposes and no PSUM->SBUF relayout copies.

Per batch b (reference):
    W_h, W_e = W_attn[:H], W_attn[H:]
    proj   = hidden @ W_h + enc[b] @ W_e + b_attn          # [S, H]
    energy = tanh(proj);  scores = energy @ W_v            # [S]
    attn   = softmax(where(mask==0, -1e10, scores))
    ctx    = attn @ enc[b]                                 # [2H]

Device dataflow (per core, 4 batches, s_p packed rows):
  projT[h,s] = sum_e W_e[e,h]^T encT[e,s]   PE, W_e chunks stationary (bf16)
  energyT    = tanh(projT + hb[h])          ACT, hb as per-partition bias
  scoresT    = sum_h wv[h] energyT[h,s]     PE, enT chunks stationary, N=1
  softmax over packed columns (pad bias -1e10), denom via ones-matmul
  ctxT[e]    = sum_s enc[s,e]^T attn[s]     PE, enc chunks stationary, N=1

Numerics: bf16 matmul datapath, f32 softmax.  rel err ~5e-3 (tol 2e-2).
"""

import math
import numpy as np
from ml_dtypes import bfloat16

B, S, H = 32, 1024, 512
E = 2 * H             # 1024
N_CORES = 8
B_LOC = B // N_CORES  # 4
HC = H // 128         # 4 output h-chunks
KE = E // 128         # 8 contraction e-chunks
KH = H // 128         # 4 contraction chunks for hidden @ W_h
NEG = -1e10

_cache = {}


def _install_tile_drain_patch():
    """walrus in this container rejects >1 sem-wait on the SP CTRL drain that
    TileContext emits at kernel tail; split the waits across 1-wait nops."""
    import concourse.tile as tile
    import concourse.mybir as mybir
    from concourse.vector_clock import ScopedClock

    if getattr(tile.TileContext, "_drain_patch_installed", False):
        return

    def _drain_and_barrier_split(self, tick_clock, wait_clock):
        nc = self.nc
        probe = nc.sync.nop(nofuse=True, hint="tail_wait_probe")
        wait_clock.add_sem_waits(
            probe.ins, ScopedClock({None: tick_clock.global_clock})
        )
        si = probe.ins.sync_info
        waits = list(si.on_wait) if si and si.on_wait else []
        if len(waits) > 1:
            si.on_wait = waits[:1]
            for w in waits[1:]:
                n = nc.sync.nop(nofuse=True, hint="tail_wait_extra")
                nsi = n.ins.sync_info
                if nsi is None:
                    n.ins.sync_info = mybir.SyncInfo(on_wait=[w], on_update=[])
                else:
                    nsi.on_wait = [w]
        nc.sync.drain()
        nc.all_engine_barrier()
        assert self.sems is not None
        popped = nc._tile_sem_poison_stack.pop()
        assert popped is self._sem_poison
        # chunked clear_and_free_semaphores: walrus rejects RANGE_CLEAR ISA
        # instructions spanning more than a few semaphores ("ISA wrong
        # length"), so clear in <=3-wide ranges.
        sems = list(self.sems.allocated().values())
        sem_nums = sorted(s.num if hasattr(s, "num") else s for s in sems)
        if sem_nums:
            runs = []
            lo = prev = sem_nums[0]
            for n in sem_nums[1:]:
                if n == prev + 1:
                    prev = n
                else:
                    runs.append((lo, prev))
                    lo = prev = n
            runs.append((lo, prev))
            for lo, hi in runs:
                for c0 in range(lo, hi + 1, 3):
                    c1 = min(c0 + 2, hi)
                    r = range(c0, c1 + 1)
                    assert nc._state.free_isdisjoint(r)
                    nc.gpsimd.dma_reset(r)
                    nc.gpsimd.sem_clear(r)
            nc._state.prepend_free_semaphores(sem_nums)
            for poison_set in nc._tile_sem_poison_stack:
                poison_set.update(sem_nums)
        nc.all_engine_barrier()

    tile.TileContext._drain_and_barrier = _drain_and_barrier_split
    tile.TileContext._drain_patch_installed = True


def _split_multiwaits(nc, max_waits=1):
    """walrus's setupSyncWait rejects instructions carrying more than a couple
    of semaphore waits.  Move excess waits onto same-engine nops inserted
    immediately before the offending instruction (engine executes in order, so
    semantics are identical)."""
    import concourse.mybir as mybir

    for f in nc.m.functions:
        for bb in f.blocks:
            out = []
            for inst in bb.instructions:
                si = inst.sync_info
                waits = list(si.on_wait) if si and si.on_wait else []
                lim = max_waits
                if len(waits) > lim:
                    excess = waits[:-lim]
                    si.on_wait = waits[-lim:]
                    for i in range(0, len(excess), max_waits):
                        nop = mybir.InstNoOp(
                            name=f"I-{nc.next_id()}-waitsplit", ins=[], outs=[]
                        )
                        nop.engine = inst.engine
                        nop.sync_info = mybir.SyncInfo(
                            on_wait=excess[i:i + max_waits], on_update=[]
                        )
                        nc.register_instruction(nop, overwrite=True)
                        out.append(nop)
                out.append(inst)
            bb.instructions[:] = out


def build_kernel(n_iters: int = 1, s_p: int = 552):
    """Per-core Bass program for packed row count s_p (multiple of 8).
    n_iters>1 repeats the compute body for slope-based timing."""
    _install_tile_drain_patch()
    import concourse.bass as bass
    import concourse.tile as tile
    import concourse.mybir as mybir
    from concourse.mybir import ActivationFunctionType as act

    f32 = mybir.dt.float32
    bf16 = mybir.dt.bfloat16

    ST = (s_p + 127) // 128          # s-tiles (last may be partial)
    R_LAST = s_p - 128 * (ST - 1)    # rows in last s-tile
    SA = min(512, s_p)               # PSUM s-split: [0,SA) + [SA,s_p)
    SB = s_p - SA
    STF = ST * 128                   # padded col count for col-layout tiles

    nc = bass.Bass("TRN2", target_bir_lowering=False, debug=False,
                   num_devices=N_CORES)

    encp_d = nc.dram_tensor("encp", [B_LOC, 128, ST * E], bf16,
                            kind="ExternalInput").ap()
    enctp_d = nc.dram_tensor("enctp", [B_LOC, 128, KE * s_p], bf16,
                             kind="ExternalInput").ap()
    wh_d = nc.dram_tensor("wh_sb", [128, KH * HC * 128], bf16,
                          kind="ExternalInput").ap()
    we_d = nc.dram_tensor("we_sb", [128, KE * HC * 128], bf16,
                          kind="ExternalInput").ap()
    hidT_d = nc.dram_tensor("hidT", [128, KH * B_LOC], bf16,
                            kind="ExternalInput").ap()
    wv_d = nc.dram_tensor("wv_col", [128, HC], bf16,
                          kind="ExternalInput").ap()
    battn_d = nc.dram_tensor("battn_row", [1, H], bf16,
                             kind="ExternalInput").ap()
    pb_d = nc.dram_tensor("pb_cols", [128, B_LOC * ST], f32,
                          kind="ExternalInput").ap()
    # column-layout outputs (one contiguous [128, ST+KE] block per batch;
    # host unpacks): out[:, 0:ST] = attn cols, out[:, ST:ST+KE] = ctxT
    out_d = nc.dram_tensor("out_cols", [B_LOC, 128, ST + KE], f32,
                           kind="ExternalOutput").ap()

    with tile.TileContext(nc) as tc:
        with (
            tc.tile_pool(name="const", bufs=1) as cpool,
            tc.tile_pool(name="enc", bufs=3) as encpool,
            tc.tile_pool(name="encT", bufs=3) as encTpool,
            tc.tile_pool(name="enrg", bufs=9) as enpool,
            tc.tile_pool(name="perb", bufs=3) as bpool,
            tc.tile_pool(name="psA", bufs=2, space="PSUM") as psA_pool,
            tc.tile_pool(name="psB", bufs=2, space="PSUM") as psB_pool,
            tc.tile_pool(name="pscol", bufs=2, space="PSUM") as pscol_pool,
            tc.tile_pool(name="pctxT", bufs=1, space="PSUM") as pctx_pool,
            tc.tile_pool(name="psm", bufs=1, space="PSUM") as psm_pool,
        ):
            # ---------------- constants / weights / small inputs ----------
            # (small tensors on the SP HWDGE ring; big ones on the ACT ring)
            hidT = cpool.tile([128, KH * B_LOC], bf16)
            nc.sync.dma_start(hidT[:], hidT_d)
            wv_col = cpool.tile([128, HC], bf16)
            nc.sync.dma_start(wv_col[:], wv_d)
            battn_row = cpool.tile([1, H], bf16)
            nc.sync.dma_start(battn_row[:], battn_d)
            pb_sb = cpool.tile([128, B_LOC * ST], f32)
            nc.sync.dma_start(pb_sb[:], pb_d)
            w_h = cpool.tile([128, KH * HC * 128], bf16)
            nc.sync.dma_start(w_h[:], wh_d)

            w_e = cpool.tile([128, KE * HC * 128], bf16)
            nc.scalar.dma_start(w_e[:], we_d)
            encT_first = encTpool.tile([128, KE * s_p], bf16, tag="encT")
            nc.scalar.dma_start(encT_first[:], enctp_d[0])
            enc_first = encpool.tile([128, ST * E], bf16, tag="enc")
            nc.sync.dma_start(enc_first[:, 0:(ST - 1) * E],
                              encp_d[0][:, 0:(ST - 1) * E])
            nc.sync.dma_start(enc_first[0:R_LAST, (ST - 1) * E:ST * E],
                              encp_d[0][0:R_LAST, (ST - 1) * E:ST * E])

            ones4 = cpool.tile([1, B_LOC], bf16)
            nc.vector.memset(ones4[:], 1.0)
            ones_col = cpool.tile([128, 1], f32)
            nc.vector.memset(ones_col[:], 1.0)
            ones_row = cpool.tile([1, 128], f32)
            nc.vector.memset(ones_row[:], 1.0)

            # hbT[:, hc*B_LOC + b] = (hidden @ W_h + b_attn)[b, hc*128:+128]
            hbT = cpool.tile([128, HC * B_LOC], f32)

            def emit_preamble():
                """PE matmuls for the per-batch tanh bias; depends only on
                the small DMAs so it runs while the big enc loads land."""
                for hc in range(HC):
                    p_ph = psm_pool.tile([128, B_LOC], f32, tag="sm")
                    for k in range(KH):
                        nc.tensor.matmul(
                            p_ph[:],
                            w_h[:, (k * HC + hc) * 128:(k * HC + hc + 1) * 128],
                            hidT[:, k * B_LOC:(k + 1) * B_LOC],
                            start=(k == 0), stop=False,
                        )
                    nc.tensor.matmul(
                        p_ph[:], battn_row[:, hc * 128:(hc + 1) * 128],
                        ones4[:], start=False, stop=True,
                    )
                    nc.any.tensor_copy(hbT[:, hc * B_LOC:(hc + 1) * B_LOC],
                                       p_ph[:])

            # ---------------- deferred emission machinery -----------------
            def emit_scores(sc):
                """Scores for a finished batch.  PSUM accumulation groups must
                be contiguous per bank (start=True wipes the whole bank), so
                each column's HC members are emitted back-to-back."""
                b, enTs, p_scol = sc
                for st in range(ST):
                    for hc in range(HC):
                        nc.tensor.matmul(
                            p_scol[:, st:st + 1],
                            enTs[hc][:, st * 128:(st + 1) * 128],
                            wv_col[:, hc:hc + 1],
                            start=(hc == 0), stop=(hc == HC - 1),
                        )

            def emit_tail(pend):
                """Softmax + context for a finished batch."""
                pb, enc_b, p_scol = pend
                sm = bpool.tile([128, ST], f32, tag="sm")
                nc.vector.tensor_add(
                    sm[:], p_scol[:], pb_sb[:, pb * ST:(pb + 1) * ST]
                )
                p_exp = bpool.tile([128, ST], f32, tag="p_exp")
                rowsum = bpool.tile([128, 1], f32, tag="rowsum")
                nc.scalar.activation(p_exp[:], sm[:], act.Exp,
                                     accum_out=rowsum[:])
                p_den = psm_pool.tile([1, 1], f32, tag="sm")
                nc.tensor.matmul(p_den[:], rowsum[:], ones_col[:],
                                 start=True, stop=True)
                rd = bpool.tile([1, 1], f32, tag="rd")
                nc.vector.reciprocal(rd[:], p_den[:])
                p_rb = psm_pool.tile([128, 1], f32, tag="sm")
                nc.tensor.matmul(p_rb[:], ones_row[:], rd[:],
                                 start=True, stop=True)
                rb = bpool.tile([128, 1], f32, tag="rb")
                nc.any.tensor_copy(rb[:], p_rb[:])
                out_sb = bpool.tile([128, ST + KE], f32, tag="out_sb")
                nc.vector.tensor_scalar_mul(out_sb[:, 0:ST], p_exp[:], rb[:])
                p_r = bpool.tile([128, ST], bf16, tag="p_r")
                nc.vector.tensor_scalar_mul(p_r[:], p_exp[:], rb[:])
                # ctxT[e] = sum_s enc[s, e] * attn[s]: enc chunks stationary
                p_ctxT = pctx_pool.tile([128, KE], f32, tag="p_ctxT")
                for ec in range(KE):
                    for st in range(ST):
                        rows = 128 if st < ST - 1 else R_LAST
                        nc.tensor.matmul(
                            p_ctxT[:, ec:ec + 1],
                            enc_b[0:rows,
                                  st * E + ec * 128:st * E + (ec + 1) * 128],
                            p_r[0:rows, st:st + 1],
                            start=(st == 0), stop=(st == ST - 1),
                        )
                nc.any.tensor_copy(out_sb[:, ST:ST + KE], p_ctxT[:])
                nc.gpsimd.dma_start(out_d[pb], out_sb[:])

            # ---------------- main loop -----------------------------------
            emit_preamble()
            pend_sc = None    # scores group awaiting emission
            pend_tail = None  # finished batch awaiting softmax+ctx
            enc_b = enc_first
            encT_b = encT_first
            enc_next = encT_next = None
            for it in range(n_iters):
                for b in range(B_LOC):
                    if not (it == 0 and b == 0):
                        enc_b, encT_b = enc_next, encT_next
                    last_batch = (it == n_iters - 1 and b == B_LOC - 1)
                    if not last_batch:
                        nb = (b + 1) % B_LOC
                        encT_next = encTpool.tile([128, KE * s_p], bf16,
                                                  tag="encT")
                        nc.scalar.dma_start(encT_next[:], enctp_d[nb])
                        enc_next = encpool.tile([128, ST * E], bf16,
                                                tag="enc")
                        nc.sync.dma_start(enc_next[:, 0:(ST - 1) * E],
                                          encp_d[nb][:, 0:(ST - 1) * E])
                        nc.sync.dma_start(
                            enc_next[0:R_LAST, (ST - 1) * E:ST * E],
                            encp_d[nb][0:R_LAST, (ST - 1) * E:ST * E])

                    p_scol = pscol_pool.tile([128, ST], f32, tag="p_scol")
                    enTs = []
                    for hc in range(HC):
                        pA = psA_pool.tile([128, SA], f32, tag="pA")
                        if SB:
                            pB = psB_pool.tile([128, SB], f32, tag="pB")
                        for k in range(KE):
                            lhs = w_e[:, (k * HC + hc) * 128:
                                      (k * HC + hc + 1) * 128]
                            nc.tensor.matmul(
                                pA[:], lhs,
                                encT_b[:, k * s_p:k * s_p + SA],
                                start=(k == 0), stop=(k == KE - 1),
                            )
                            if SB:
                                nc.tensor.matmul(
                                    pB[:], lhs,
                                    encT_b[:, k * s_p + SA:(k + 1) * s_p],
                                    start=(k == 0), stop=(k == KE - 1),
                                )
                        if hc == 1 and pend_sc is not None:
                            emit_scores(pend_sc)
                            pend_sc = None
                        if hc == 2 and pend_tail is not None:
                            emit_tail(pend_tail)
                            pend_tail = None
                        # energyT = tanh(projT + hb), padded cols zeroed so
                        # the full-width scores matmul stays NaN-free
                        enT = enpool.tile([128, STF], bf16, tag="enT")
                        if STF > s_p:
                            nc.vector.memset(enT[:, s_p:STF], 0.0)
                        hb_col = hbT[:, hc * B_LOC + b:hc * B_LOC + b + 1]
                        nc.scalar.activation(enT[:, 0:SA], pA[:], act.Tanh,
                                             bias=hb_col)
                        if SB:
                            nc.scalar.activation(enT[:, SA:s_p], pB[:],
                                                 act.Tanh, bias=hb_col)
                        enTs.append(enT)
                    pend_sc = (b, enTs, p_scol)
                    pend_tail_next = (b, enc_b, p_scol)
                    if pend_tail is not None:
                        emit_tail(pend_tail)
                    pend_tail = pend_tail_next
                # end b loop
            if pend_sc is not None:
                emit_scores(pend_sc)
                pend_sc = None
            if pend_tail is not None:
                emit_tail(pend_tail)
                pend_tail = None

    _split_multiwaits(nc)
    import concourse.mybir as mybir2
    mybir2.codegen_inst_isa_subclasses(nc)
    return nc


def _get_nc(n_iters: int = 1, s_p: int = 552):
    key = ("nc", n_iters, s_p)
    if key not in _cache:
        _cache[key] = build_kernel(n_iters, s_p)
    return _cache[key]


def pick_s_p(mask):
    counts = mask.reshape(B, S).sum(1)
    m = int(counts.max())
    return max(128, min(S, ((m + 7) // 8) * 8))


def _pack_core(enc_c, mask_c, s_p):
    """Host-side pack: mask-selected rows in tile layout (both orientations,
    bf16) + pad bias columns + scatter info."""
    ST = (s_p + 127) // 128
    encp = np.zeros((B_LOC, 128, ST * E), bfloat16)
    enctp = np.zeros((B_LOC, 128, KE * s_p), bfloat16)
    pb_pad = np.full((B_LOC, ST * 128), NEG, np.float32)
    scat = []
    for b in range(B_LOC):
        idx = np.nonzero(mask_c[b])[0]
        n = len(idx)
        rows = enc_c[b, idx].astype(bfloat16)          # [n, E]
        buf = np.zeros((ST * 128, E), bfloat16)
        buf[:n] = rows
        encp[b] = buf.reshape(ST, 128, E).transpose(1, 0, 2).reshape(
            128, ST * E)
        t = np.zeros((s_p, E), bfloat16)
        t[:n] = rows
        enctp[b] = t.T.reshape(KE, 128, s_p).transpose(1, 0, 2).reshape(
            128, KE * s_p)
        pb_pad[b, :n] = 0.0
        scat.append((n, idx.astype(np.int64)))
    pb_cols = pb_pad.reshape(B_LOC, ST, 128).transpose(2, 0, 1).reshape(
        128, B_LOC * ST)
    return encp, enctp, np.ascontiguousarray(pb_cols), scat


def shard_inputs(hidden, encoder_outputs, mask, W_attn, b_attn, W_v,
                 s_p=None):
    hidden = np.asarray(hidden, dtype=np.float32)
    enc = np.asarray(encoder_outputs, dtype=np.float32)
    mask = np.asarray(mask, dtype=np.int32)
    W_attn = np.asarray(W_attn, dtype=np.float32)
    b_attn = np.asarray(b_attn, dtype=np.float32)
    W_v = np.asarray(W_v, dtype=np.float32)
    if s_p is None:
        s_p = pick_s_p(mask)

    w_h = W_attn[:H].astype(bfloat16)                  # [512, 512]
    w_e = W_attn[H:].astype(bfloat16)                  # [1024, 512]
    wh_sb = np.ascontiguousarray(
        w_h.reshape(KH, 128, HC, 128).transpose(1, 0, 2, 3).reshape(
            128, KH * HC * 128))
    we_sb = np.ascontiguousarray(
        w_e.reshape(KE, 128, HC, 128).transpose(1, 0, 2, 3).reshape(
            128, KE * HC * 128))
    wv_col = np.ascontiguousarray(
        W_v.astype(bfloat16).reshape(HC, 128).T)       # [128, HC]
    battn_row = np.ascontiguousarray(
        b_attn.astype(bfloat16)[None, :])              # [1, H]

    in_maps, scats = [], []
    for c in range(N_CORES):
        sl = slice(c * B_LOC, (c + 1) * B_LOC)
        hidT = np.ascontiguousarray(
            hidden[sl].astype(bfloat16).T.reshape(
                KH, 128, B_LOC).transpose(1, 0, 2).reshape(128, KH * B_LOC))
        encp, enctp, pb_cols, scat = _pack_core(enc[sl], mask[sl], s_p)
        in_maps.append({
            "encp": encp,
            "enctp": enctp,
            "wh_sb": wh_sb,
            "we_sb": we_sb,
            "hidT": hidT,
            "wv_col": wv_col,
            "battn_row": battn_row,
            "pb_cols": pb_cols,
        })
        scats.append(scat)
    return in_maps, scats, s_p


def kernel(hidden, encoder_outputs, mask, W_attn, b_attn, W_v):
    from concourse.bass_utils import run_bass_kernel_spmd

    in_maps, scats, s_p = shard_inputs(
        hidden, encoder_outputs, mask, W_attn, b_attn, W_v)
    nc = _get_nc(1, s_p)
    res = run_bass_kernel_spmd(nc, in_maps, list(range(N_CORES)))
    ST = (s_p + 127) // 128
    context = np.zeros((B, E), np.float32)
    attn_w = np.zeros((B, S), np.float32)
    for c in range(N_CORES):
        out = res.results[c]["out_cols"]          # [B_LOC, 128, ST+KE]
        for b in range(B_LOC):
            n, idx = scats[c][b]
            attn_p = out[b, :, 0:ST].T.reshape(-1)[:n]   # s = st*128 + p
            attn_w[c * B_LOC + b, idx] = attn_p
            context[c * B_LOC + b] = out[b, :, ST:].T.reshape(-1)  # e = ec*128+p
    return context, attn_w


# revision 17
# speedup vs baseline: 7.7542x; 2.0268x over previous
"""Bahdanau-style attention kernel for Trainium2, data-parallel over batch
across 8 NeuronCores.  v5: the host pre-packs the mask-selected encoder rows
(exp(-1e10+x) underflows to 0, so dropped rows are exact), pre-casts to bf16,
and uploads ONLY the transposed layout encT [e-part, s].  The projection uses
encT as the moving operand with W_e chunks stationary; the context matmul is
computed on the (otherwise idle) vector engine as a fused multiply-reduce of
encT chunks against a PE-broadcast attention row, so the s-major enc layout is
never needed and HBM traffic is halved.

Per batch b (reference):
    W_h, W_e = W_attn[:H], W_attn[H:]
    proj   = hidden @ W_h + enc[b] @ W_e + b_attn          # [S, H]
    energy = tanh(proj);  scores = energy @ W_v            # [S]
    attn   = softmax(where(mask==0, -1e10, scores))
    ctx    = attn @ enc[b]                                 # [2H]

Device dataflow (per core, 4 batches, s_p packed rows):
  projT[h,s] = sum_e W_e[e,h]^T encT[e,s]   PE, W_e chunks stationary (bf16)
  energyT    = tanh(projT + hb[h])          ACT, hb as per-partition bias
  scores     = sum_h wv[h] energyT[h,s]     PE, wv stationary, [1,s] row out
  softmax on the [1, s_p] row (pad bias -1e10), denom via exp accum_out
  attn_bc    = ones^T attn_row              PE broadcast to 128 partitions
  ctxT[e]    = reduce_s(encT[e,s]*attn_bc)  DVE tensor_tensor_reduce per chunk

Numerics: bf16 matmul datapath, f32 softmax.  rel err ~3e-3 (tol 2e-2).
"""

import math
import numpy as np
from ml_dtypes import bfloat16

B, S, H = 32, 1024, 512
E = 2 * H             # 1024
N_CORES = 8
B_LOC = B // N_CORES  # 4
HC = H // 128         # 4 output h-chunks
KE = E // 128         # 8 contraction e-chunks
KH = H // 128         # 4 contraction chunks for hidden @ W_h
NEG = -1e10

_cache = {}


def _install_tile_drain_patch():
    """walrus in this container rejects >1 sem-wait on the SP CTRL drain that
    TileContext emits at kernel tail; split the waits across 1-wait nops."""
    import concourse.tile as tile
    import concourse.mybir as mybir
    from concourse.vector_clock import ScopedClock

    if getattr(tile.TileContext, "_drain_patch_installed", False):
        return

    def _drain_and_barrier_split(self, tick_clock, wait_clock):
        nc = self.nc
        probe = nc.sync.nop(nofuse=True, hint="tail_wait_probe")
        wait_clock.add_sem_waits(
            probe.ins, ScopedClock({None: tick_clock.global_clock})
        )
        si = probe.ins.sync_info
        waits = list(si.on_wait) if si and si.on_wait else []
        if len(waits) > 1:
            si.on_wait = waits[:1]
            for w in waits[1:]:
                n = nc.sync.nop(nofuse=True, hint="tail_wait_extra")
                nsi = n.ins.sync_info
                if nsi is None:
                    n.ins.sync_info = mybir.SyncInfo(on_wait=[w], on_update=[])
                else:
                    nsi.on_wait = [w]
        nc.sync.drain()
        nc.all_engine_barrier()
        assert self.sems is not None
        popped = nc._tile_sem_poison_stack.pop()
        assert popped is self._sem_poison
        # chunked clear_and_free_semaphores: walrus rejects RANGE_CLEAR ISA
        # instructions spanning more than a few semaphores ("ISA wrong
        # length"), so clear in <=3-wide ranges.
        sems = list(self.sems.allocated().values())
        sem_nums = sorted(s.num if hasattr(s, "num") else s for s in sems)
        if sem_nums:
            runs = []
            lo = prev = sem_nums[0]
            for n in sem_nums[1:]:
                if n == prev + 1:
                    prev = n
                else:
                    runs.append((lo, prev))
                    lo = prev = n
            runs.append((lo, prev))
            for lo, hi in runs:
                for c0 in range(lo, hi + 1, 3):
                    c1 = min(c0 + 2, hi)
                    r = range(c0, c1 + 1)
                    assert nc._state.free_isdisjoint(r)
                    nc.gpsimd.dma_reset(r)
                    nc.gpsimd.sem_clear(r)
            nc._state.prepend_free_semaphores(sem_nums)
            for poison_set in nc._tile_sem_poison_stack:
                poison_set.update(sem_nums)
        nc.all_engine_barrier()

    tile.TileContext._drain_and_barrier = _drain_and_barrier_split
    tile.TileContext._drain_patch_installed = True


def _split_multiwaits(nc, max_waits=1):
    """walrus's setupSyncWait rejects instructions carrying more than a couple
    of semaphore waits.  Move excess waits onto same-engine nops inserted
    immediately before the offending instruction (engine executes in order, so
    semantics are identical)."""
    import concourse.mybir as mybir

    for f in nc.m.functions:
        for bb in f.blocks:
            out = []
            for inst in bb.instructions:
                si = inst.sync_info
                waits = list(si.on_wait) if si and si.on_wait else []
                lim = max_waits
                if len(waits) > lim:
                    excess = waits[:-lim]
                    si.on_wait = waits[-lim:]
                    for i in range(0, len(excess), max_waits):
                        nop = mybir.InstNoOp(
                            name=f"I-{nc.next_id()}-waitsplit", ins=[], outs=[]
                        )
                        nop.engine = inst.engine
                        nop.sync_info = mybir.SyncInfo(
                            on_wait=excess[i:i + max_waits], on_update=[]
                        )
                        nc.register_instruction(nop, overwrite=True)
                        out.append(nop)
                out.append(inst)
            bb.instructions[:] = out


def build_kernel(n_iters: int = 1, s_p: int = 552):
    """Per-core Bass program for packed row count s_p (multiple of 8).
    n_iters>1 repeats the compute body for slope-based timing."""
    _install_tile_drain_patch()
    import concourse.bass as bass
    import concourse.tile as tile
    import concourse.mybir as mybir
    from concourse.mybir import ActivationFunctionType as act
    from concourse.mybir import AluOpType as alu

    f32 = mybir.dt.float32
    bf16 = mybir.dt.bfloat16

    SA = min(512, s_p)               # PSUM s-split: [0,SA) + [SA,s_p)
    SB = s_p - SA

    nc = bass.Bass("TRN2", target_bir_lowering=False, debug=False,
                   num_devices=N_CORES)

    enctp_d = nc.dram_tensor("enctp", [B_LOC, 128, KE * s_p], bf16,
                             kind="ExternalInput").ap()
    wh_d = nc.dram_tensor("wh_sb", [128, KH * HC * 128], bf16,
                          kind="ExternalInput").ap()
    we_d = nc.dram_tensor("we_sb", [128, KE * HC * 128], bf16,
                          kind="ExternalInput").ap()
    hidT_d = nc.dram_tensor("hidT", [128, KH * B_LOC], bf16,
                            kind="ExternalInput").ap()
    wv_d = nc.dram_tensor("wv_col", [128, HC], bf16,
                          kind="ExternalInput").ap()
    battn_d = nc.dram_tensor("battn_row", [1, H], bf16,
                             kind="ExternalInput").ap()
    pb_d = nc.dram_tensor("pb_rows", [1, B_LOC * s_p], f32,
                          kind="ExternalInput").ap()
    attn_d = nc.dram_tensor("out_attn", [B_LOC, s_p], f32,
                            kind="ExternalOutput").ap()
    ctx_d = nc.dram_tensor("out_ctxcols", [B_LOC, 128, KE], f32,
                           kind="ExternalOutput").ap()

    with tile.TileContext(nc) as tc:
        with (
            tc.tile_pool(name="const", bufs=1) as cpool,
            tc.tile_pool(name="encT", bufs=3) as encTpool,
            tc.tile_pool(name="enrg", bufs=9) as enpool,
            tc.tile_pool(name="perb", bufs=3) as bpool,
            tc.tile_pool(name="psA", bufs=2, space="PSUM") as psA_pool,
            tc.tile_pool(name="psB", bufs=2, space="PSUM") as psB_pool,
            tc.tile_pool(name="psr", bufs=2, space="PSUM") as psr_pool,
            tc.tile_pool(name="psm", bufs=1, space="PSUM") as psm_pool,
        ):
            # ---------------- constants / weights / small inputs ----------
            # (small tensors on the SP HWDGE ring; big ones on the ACT ring)
            hidT = cpool.tile([128, KH * B_LOC], bf16)
            nc.sync.dma_start(hidT[:], hidT_d)
            wv_col = cpool.tile([128, HC], bf16)
            nc.sync.dma_start(wv_col[:], wv_d)
            battn_row = cpool.tile([1, H], bf16)
            nc.sync.dma_start(battn_row[:], battn_d)
            pb_sb = cpool.tile([1, B_LOC * s_p], f32)
            nc.sync.dma_start(pb_sb[:], pb_d)
            w_h = cpool.tile([128, KH * HC * 128], bf16)
            nc.sync.dma_start(w_h[:], wh_d)

            w_e = cpool.tile([128, KE * HC * 128], bf16)
            nc.scalar.dma_start(w_e[:], we_d)
            encT_first = encTpool.tile([128, KE * s_p], bf16, tag="encT")
            nc.scalar.dma_start(encT_first[:], enctp_d[0])

            ones4 = cpool.tile([1, B_LOC], bf16)
            nc.vector.memset(ones4[:], 1.0)
            ones1r = cpool.tile([1, 128], bf16)
            nc.vector.memset(ones1r[:], 1.0)

            # hbT[:, hc*B_LOC + b] = (hidden @ W_h + b_attn)[b, hc*128:+128]
            hbT = cpool.tile([128, HC * B_LOC], f32)

            def emit_preamble():
                """PE matmuls for the per-batch tanh bias; depends only on
                the small DMAs so it runs while the big enc loads land."""
                for hc in range(HC):
                    p_ph = psm_pool.tile([128, B_LOC], f32, tag="sm")
                    for k in range(KH):
                        nc.tensor.matmul(
                            p_ph[:],
                            w_h[:, (k * HC + hc) * 128:(k * HC + hc + 1) * 128],
                            hidT[:, k * B_LOC:(k + 1) * B_LOC],
                            start=(k == 0), stop=False,
                        )
                    nc.tensor.matmul(
                        p_ph[:], battn_row[:, hc * 128:(hc + 1) * 128],
                        ones4[:], start=False, stop=True,
                    )
                    nc.any.tensor_copy(hbT[:, hc * B_LOC:(hc + 1) * B_LOC],
                                       p_ph[:])

            # ---------------- deferred emission machinery -----------------
            def emit_scores(sc):
                """Score-row matmuls for a finished batch: wv chunk stationary,
                energyT moving.  Accumulation groups (over hc) are contiguous
                per PSUM bank (start=True wipes the whole bank)."""
                b, enTs, p_srA, p_srB = sc
                for hc in range(HC):
                    nc.tensor.matmul(
                        p_srA[:], wv_col[:, hc:hc + 1], enTs[hc][:, 0:SA],
                        start=(hc == 0), stop=(hc == HC - 1),
                    )
                if SB:
                    for hc in range(HC):
                        nc.tensor.matmul(
                            p_srB[:], wv_col[:, hc:hc + 1],
                            enTs[hc][:, SA:s_p],
                            start=(hc == 0), stop=(hc == HC - 1),
                        )

            def emit_tail_a(pend):
                """Row softmax for a finished batch (DVE/ACT latency chain)."""
                pb, encT_b, p_srA, p_srB, t = pend
                sm = bpool.tile([1, s_p], f32, tag="sm")
                nc.vector.tensor_add(
                    sm[:, 0:SA], p_srA[:],
                    pb_sb[:, pb * s_p:pb * s_p + SA])
                if SB:
                    nc.vector.tensor_add(
                        sm[:, SA:s_p], p_srB[:],
                        pb_sb[:, pb * s_p + SA:(pb + 1) * s_p])
                p_exp = bpool.tile([1, s_p], f32, tag="p_exp")
                den = bpool.tile([1, 1], f32, tag="den")
                nc.scalar.activation(p_exp[:], sm[:], act.Exp,
                                     accum_out=den[:])
                rd = bpool.tile([1, 1], f32, tag="rd")
                nc.vector.reciprocal(rd[:], den[:])
                attn_row = bpool.tile([1, s_p], f32, tag="attn_row")
                nc.vector.tensor_scalar_mul(attn_row[:], p_exp[:], rd[:])
                nc.sync.dma_start(attn_d[pb][None, :], attn_row[:])
                attn_bf = bpool.tile([1, s_p], bf16, tag="attn_bf")
                nc.vector.tensor_scalar_mul(attn_bf[:], p_exp[:], rd[:])
                t.append(attn_bf)

            def emit_tail_b(pend):
                """Context for a finished batch: PE broadcast of the attention
                row, then fused multiply-reduce per e-chunk on the DVE."""
                pb, encT_b, p_srA, p_srB, t = pend
                attn_bf = t[0]
                bcA = psA_pool.tile([128, SA], f32, tag="pA")
                nc.tensor.matmul(bcA[:], ones1r[:], attn_bf[:, 0:SA],
                                 start=True, stop=True)
                if SB:
                    bcB = psB_pool.tile([128, SB], f32, tag="pB")
                    nc.tensor.matmul(bcB[:], ones1r[:], attn_bf[:, SA:s_p],
                                     start=True, stop=True)
                abf = bpool.tile([128, s_p], bf16, tag="abf")
                nc.vector.tensor_copy(abf[:, 0:SA], bcA[:])
                if SB:
                    nc.vector.tensor_copy(abf[:, SA:s_p], bcB[:])
                # (tensor_tensor_reduce hard-crashes this container's
                # runtime, so use the two-op mult + reduce_sum form)
                ctx_sb = bpool.tile([128, KE], f32, tag="ctx_sb")
                for ec in range(KE):
                    scratch = bpool.tile([128, s_p], bf16, tag="scratch")
                    nc.vector.tensor_mul(
                        scratch[:], encT_b[:, ec * s_p:(ec + 1) * s_p], abf[:])
                    nc.vector.reduce_sum(ctx_sb[:, ec:ec + 1], scratch[:],
                                         axis=mybir.AxisListType.X)
                nc.gpsimd.dma_start(ctx_d[pb], ctx_sb[:])

            # ---------------- main loop -----------------------------------
            emit_preamble()
            pend_sc = None    # scores awaiting emission
            pend_tail = None  # finished batch awaiting softmax+ctx
            encT_b = encT_first
            encT_next = None
            for it in range(n_iters):
                for b in range(B_LOC):
                    if not (it == 0 and b == 0):
                        encT_b = encT_next
                    last_batch = (it == n_iters - 1 and b == B_LOC - 1)
                    if not last_batch:
                        nb = (b + 1) % B_LOC
                        encT_next = encTpool.tile([128, KE * s_p], bf16,
                                                  tag="encT")
                        nc.scalar.dma_start(encT_next[:], enctp_d[nb])

                    p_srA = psr_pool.tile([1, SA], f32, tag="sr")
                    if SB:
                        p_srB = psr_pool.tile([1, SB], f32, tag="sr")
                    else:
                        p_srB = None
                    enTs = []
                    for hc in range(HC):
                        pA = psA_pool.tile([128, SA], f32, tag="pA")
                        if SB:
                            pB = psB_pool.tile([128, SB], f32, tag="pB")
                        for k in range(KE):
                            lhs = w_e[:, (k * HC + hc) * 128:
                                      (k * HC + hc + 1) * 128]
                            nc.tensor.matmul(
                                pA[:], lhs,
                                encT_b[:, k * s_p:k * s_p + SA],
                                start=(k == 0), stop=(k == KE - 1),
                            )
                            if SB:
                                nc.tensor.matmul(
                                    pB[:], lhs,
                                    encT_b[:, k * s_p + SA:(k + 1) * s_p],
                                    start=(k == 0), stop=(k == KE - 1),
                                )
                        if hc == 1 and pend_sc is not None:
                            emit_scores(pend_sc)
                            pend_sc = None
                        if hc == 2 and pend_tail is not None:
                            emit_tail_a(pend_tail)
                        if hc == 3 and pend_tail is not None:
                            emit_tail_b(pend_tail)
                            pend_tail = None
                        enT = enpool.tile([128, s_p], bf16, tag="enT")
                        hb_col = hbT[:, hc * B_LOC + b:hc * B_LOC + b + 1]
                        nc.scalar.activation(enT[:, 0:SA], pA[:], act.Tanh,
                                             bias=hb_col)
                        if SB:
                            nc.scalar.activation(enT[:, SA:s_p], pB[:],
                                                 act.Tanh, bias=hb_col)
                        enTs.append(enT)
                    pend_sc = (b, enTs, p_srA, p_srB)
                    if pend_tail is not None:
                        emit_tail_a(pend_tail)
                        emit_tail_b(pend_tail)
                    pend_tail = (b, encT_b, p_srA, p_srB, [])
                # end b loop
            if pend_sc is not None:
                emit_scores(pend_sc)
                pend_sc = None
            if pend_tail is not None:
                emit_tail_a(pend_tail)
                emit_tail_b(pend_tail)
                pend_tail = None

    _split_multiwaits(nc)
    import concourse.mybir as mybir2
    mybir2.codegen_inst_isa_subclasses(nc)
    return nc


def _get_nc(n_iters: int = 1, s_p: int = 552):
    key = ("nc", n_iters, s_p)
    if key not in _cache:
        _cache[key] = build_kernel(n_iters, s_p)
    return _cache[key]


def pick_s_p(mask):
    counts = mask.reshape(B, S).sum(1)
    m = int(counts.max())
    return max(128, min(S, ((m + 7) // 8) * 8))


def _pack_core(enc_c, mask_c, s_p):
    """Host-side pack: mask-selected rows, e-major bf16 tile layout + pad
    bias rows + scatter info."""
    enctp = np.zeros((B_LOC, 128, KE * s_p), bfloat16)
    pbias = np.zeros((B_LOC, s_p), np.float32)
    scat = []
    for b in range(B_LOC):
        idx = np.nonzero(mask_c[b])[0]
        n = len(idx)
        rows = enc_c[b, idx].astype(bfloat16)          # [n, E]
        t = np.zeros((s_p, E), bfloat16)
        t[:n] = rows
        enctp[b] = t.T.reshape(KE, 128, s_p).transpose(1, 0, 2).reshape(
            128, KE * s_p)
        pbias[b, n:] = NEG
        scat.append((n, idx.astype(np.int64)))
    return enctp, pbias.reshape(1, -1), scat


def shard_inputs(hidden, encoder_outputs, mask, W_attn, b_attn, W_v,
                 s_p=None):
    hidden = np.asarray(hidden, dtype=np.float32)
    enc = np.asarray(encoder_outputs, dtype=np.float32)
    mask = np.asarray(mask, dtype=np.int32)
    W_attn = np.asarray(W_attn, dtype=np.float32)
    b_attn = np.asarray(b_attn, dtype=np.float32)
    W_v = np.asarray(W_v, dtype=np.float32)
    if s_p is None:
        s_p = pick_s_p(mask)

    w_h = W_attn[:H].astype(bfloat16)                  # [512, 512]
    w_e = W_attn[H:].astype(bfloat16)                  # [1024, 512]
    wh_sb = np.ascontiguousarray(
        w_h.reshape(KH, 128, HC, 128).transpose(1, 0, 2, 3).reshape(
            128, KH * HC * 128))
    we_sb = np.ascontiguousarray(
        w_e.reshape(KE, 128, HC, 128).transpose(1, 0, 2, 3).reshape(
            128, KE * HC * 128))
    wv_col = np.ascontiguousarray(
        W_v.astype(bfloat16).reshape(HC, 128).T)       # [128, HC]
    battn_row = np.ascontiguousarray(
        b_attn.astype(bfloat16)[None, :])              # [1, H]

    in_maps, scats = [], []
    for c in range(N_CORES):
        sl = slice(c * B_LOC, (c + 1) * B_LOC)
        hidT = np.ascontiguousarray(
            hidden[sl].astype(bfloat16).T.reshape(
                KH, 128, B_LOC).transpose(1, 0, 2).reshape(128, KH * B_LOC))
        enctp, pb_rows, scat = _pack_core(enc[sl], mask[sl], s_p)
        in_maps.append({
            "enctp": enctp,
            "wh_sb": wh_sb,
            "we_sb": we_sb,
            "hidT": hidT,
            "wv_col": wv_col,
            "battn_row": battn_row,
            "pb_rows": pb_rows,
        })
        scats.append(scat)
    return in_maps, scats, s_p


def kernel(hidden, encoder_outputs, mask, W_attn, b_attn, W_v):
    from concourse.bass_utils import run_bass_kernel_spmd

    in_maps, scats, s_p = shard_inputs(
        hidden, encoder_outputs, mask, W_attn, b_attn, W_v)
    nc = _get_nc(1, s_p)
    res = run_bass_kernel_spmd(nc, in_maps, list(range(N_CORES)))
    context = np.zeros((B, E), np.float32)
    attn_w = np.zeros((B, S), np.float32)
    for c in range(N_CORES):
        attn_p = res.results[c]["out_attn"]          # [B_LOC, s_p]
        ctx_c = res.results[c]["out_ctxcols"]        # [B_LOC, 128, KE]
        for b in range(B_LOC):
            n, idx = scats[c][b]
            attn_w[c * B_LOC + b, idx] = attn_p[b, :n]
            context[c * B_LOC + b] = ctx_c[b].T.reshape(-1)  # e = ec*128+p
    return context, attn_w


# revision 21
# speedup vs baseline: 10.5860x; 1.3652x over previous
"""Bahdanau-style attention kernel for Trainium2, data-parallel over batch
across 8 NeuronCores.  v5: the host pre-packs the mask-selected encoder rows
(exp(-1e10+x) underflows to 0, so dropped rows are exact), pre-casts to bf16,
and uploads ONLY the transposed layout encT [e-part, s].  The projection uses
encT as the moving operand with W_e chunks stationary; the context matmul is
computed on the (otherwise idle) vector engine as a fused multiply-reduce of
encT chunks against a PE-broadcast attention row, so the s-major enc layout is
never needed and HBM traffic is halved.

Per batch b (reference):
    W_h, W_e = W_attn[:H], W_attn[H:]
    proj   = hidden @ W_h + enc[b] @ W_e + b_attn          # [S, H]
    energy = tanh(proj);  scores = energy @ W_v            # [S]
    attn   = softmax(where(mask==0, -1e10, scores))
    ctx    = attn @ enc[b]                                 # [2H]

Device dataflow (per core, 4 batches, s_p packed rows):
  projT[h,s] = sum_e W_e[e,h]^T encT[e,s]   PE, W_e chunks stationary (bf16)
  energyT    = tanh(projT + hb[h])          ACT, hb as per-partition bias
  scores     = sum_h wv[h] energyT[h,s]     PE, wv stationary, [1,s] row out
  softmax on the [1, s_p] row (pad bias -1e10), denom via exp accum_out
  attn_bc    = ones^T attn_row              PE broadcast to 128 partitions
  ctxT[e]    = reduce_s(encT[e,s]*attn_bc)  DVE mult + DVE/ACT reduce per chunk

Numerics: bf16 matmul datapath, f32 softmax.  rel err ~3e-3 (tol 2e-2).
"""

import numpy as np
from ml_dtypes import bfloat16

B, S, H = 32, 1024, 512
E = 2 * H             # 1024
N_CORES = 8
B_LOC = B // N_CORES  # 4
HC = H // 128         # 4 output h-chunks
KE = E // 128         # 8 contraction e-chunks
KH = H // 128         # 4 contraction chunks for hidden @ W_h
NEG = -1e10

_cache = {}


def _install_tile_drain_patch():
    """walrus in this container rejects >1 sem-wait on the SP CTRL drain that
    TileContext emits at kernel tail; split the waits across 1-wait nops."""
    import concourse.tile as tile
    import concourse.mybir as mybir
    from concourse.vector_clock import ScopedClock

    if getattr(tile.TileContext, "_drain_patch_installed", False):
        return

    def _drain_and_barrier_split(self, tick_clock, wait_clock):
        nc = self.nc
        probe = nc.sync.nop(nofuse=True, hint="tail_wait_probe")
        wait_clock.add_sem_waits(
            probe.ins, ScopedClock({None: tick_clock.global_clock})
        )
        si = probe.ins.sync_info
        waits = list(si.on_wait) if si and si.on_wait else []
        if len(waits) > 1:
            si.on_wait = waits[:1]
            for w in waits[1:]:
                n = nc.sync.nop(nofuse=True, hint="tail_wait_extra")
                nsi = n.ins.sync_info
                if nsi is None:
                    n.ins.sync_info = mybir.SyncInfo(on_wait=[w], on_update=[])
                else:
                    nsi.on_wait = [w]
        nc.sync.drain()
        nc.all_engine_barrier()
        assert self.sems is not None
        popped = nc._tile_sem_poison_stack.pop()
        assert popped is self._sem_poison
        # chunked clear_and_free_semaphores: walrus rejects RANGE_CLEAR ISA
        # instructions spanning more than a few semaphores ("ISA wrong
        # length"), so clear in <=3-wide ranges.
        sems = list(self.sems.allocated().values())
        sem_nums = sorted(s.num if hasattr(s, "num") else s for s in sems)
        if sem_nums:
            runs = []
            lo = prev = sem_nums[0]
            for n in sem_nums[1:]:
                if n == prev + 1:
                    prev = n
                else:
                    runs.append((lo, prev))
                    lo = prev = n
            runs.append((lo, prev))
            for lo, hi in runs:
                for c0 in range(lo, hi + 1, 3):
                    c1 = min(c0 + 2, hi)
                    r = range(c0, c1 + 1)
                    assert nc._state.free_isdisjoint(r)
                    nc.gpsimd.dma_reset(r)
                    nc.gpsimd.sem_clear(r)
            nc._state.prepend_free_semaphores(sem_nums)
            for poison_set in nc._tile_sem_poison_stack:
                poison_set.update(sem_nums)
        nc.all_engine_barrier()

    tile.TileContext._drain_and_barrier = _drain_and_barrier_split
    tile.TileContext._drain_patch_installed = True


def _split_multiwaits(nc, max_waits=1):
    """walrus's setupSyncWait rejects instructions carrying more than a couple
    of semaphore waits.  Move excess waits onto same-engine nops inserted
    immediately before the offending instruction (engine executes in order, so
    semantics are identical)."""
    import concourse.mybir as mybir

    for f in nc.m.functions:
        for bb in f.blocks:
            out = []
            for inst in bb.instructions:
                si = inst.sync_info
                waits = list(si.on_wait) if si and si.on_wait else []
                lim = max_waits
                if len(waits) > lim:
                    excess = waits[:-lim]
                    si.on_wait = waits[-lim:]
                    for i in range(0, len(excess), max_waits):
                        nop = mybir.InstNoOp(
                            name=f"I-{nc.next_id()}-waitsplit", ins=[], outs=[]
                        )
                        nop.engine = inst.engine
                        nop.sync_info = mybir.SyncInfo(
                            on_wait=excess[i:i + max_waits], on_update=[]
                        )
                        nc.register_instruction(nop, overwrite=True)
                        out.append(nop)
                out.append(inst)
            bb.instructions[:] = out


def build_kernel(n_iters: int = 1, s_p: int = 552):
    """Per-core Bass program for packed row count s_p (multiple of 8).
    n_iters>1 repeats the compute body for slope-based timing."""
    _install_tile_drain_patch()
    import concourse.bass as bass
    import concourse.tile as tile
    import concourse.mybir as mybir
    from concourse.mybir import ActivationFunctionType as act

    f32 = mybir.dt.float32
    bf16 = mybir.dt.bfloat16

    SA = min(512, s_p)               # PSUM s-split: [0,SA) + [SA,s_p)
    SB = s_p - SA

    nc = bass.Bass("TRN2", target_bir_lowering=False, debug=False,
                   num_devices=N_CORES)

    enctp_d = nc.dram_tensor("enctp", [B_LOC, 128, KE * s_p], bf16,
                             kind="ExternalInput").ap()
    wh_d = nc.dram_tensor("wh_sb", [128, KH * HC * 128], bf16,
                          kind="ExternalInput").ap()
    we_d = nc.dram_tensor("we_sb", [128, KE * HC * 128], bf16,
                          kind="ExternalInput").ap()
    hidT_d = nc.dram_tensor("hidT", [128, KH * B_LOC], bf16,
                            kind="ExternalInput").ap()
    wv_d = nc.dram_tensor("wv_col", [128, HC], bf16,
                          kind="ExternalInput").ap()
    battn_d = nc.dram_tensor("battn_row", [1, H], bf16,
                             kind="ExternalInput").ap()
    pb_d = nc.dram_tensor("pb_rows", [1, B_LOC * s_p], f32,
                          kind="ExternalInput").ap()
    attn_d = nc.dram_tensor("out_attn", [B_LOC, s_p], f32,
                            kind="ExternalOutput").ap()
    ctx_d = nc.dram_tensor("out_ctxcols", [B_LOC, 128, KE], f32,
                           kind="ExternalOutput").ap()

    with tile.TileContext(nc) as tc:
        with (
            tc.tile_pool(name="const", bufs=1) as cpool,
            tc.tile_pool(name="encT", bufs=3) as encTpool,
            tc.tile_pool(name="enrg", bufs=9) as enpool,
            tc.tile_pool(name="perb", bufs=3) as bpool,
            tc.tile_pool(name="psA", bufs=2, space="PSUM") as psA_pool,
            tc.tile_pool(name="psB", bufs=2, space="PSUM") as psB_pool,
            tc.tile_pool(name="psr", bufs=2, space="PSUM") as psr_pool,
            tc.tile_pool(name="psm", bufs=1, space="PSUM") as psm_pool,
        ):
            # ---------------- constants / weights / small inputs ----------
            # (small tensors on the SP HWDGE ring; big ones on the ACT ring)
            hidT = cpool.tile([128, KH * B_LOC], bf16)
            nc.sync.dma_start(hidT[:], hidT_d)
            wv_col = cpool.tile([128, HC], bf16)
            nc.sync.dma_start(wv_col[:], wv_d)
            battn_row = cpool.tile([1, H], bf16)
            nc.sync.dma_start(battn_row[:], battn_d)
            pb_sb = cpool.tile([1, B_LOC * s_p], f32)
            nc.sync.dma_start(pb_sb[:], pb_d)
            w_h = cpool.tile([128, KH * HC * 128], bf16)
            nc.sync.dma_start(w_h[:], wh_d)

            w_e = cpool.tile([128, KE * HC * 128], bf16)
            nc.scalar.dma_start(w_e[:], we_d)
            encT_first = encTpool.tile([128, KE * s_p], bf16, tag="encT")
            nc.scalar.dma_start(encT_first[:], enctp_d[0])

            ones4 = cpool.tile([1, B_LOC], bf16)
            nc.vector.memset(ones4[:], 1.0)
            ones1r = cpool.tile([1, 128], bf16)
            nc.vector.memset(ones1r[:], 1.0)

            # hbT[:, hc*B_LOC + b] = (hidden @ W_h + b_attn)[b, hc*128:+128]
            hbT = cpool.tile([128, HC * B_LOC], f32)

            def emit_preamble():
                """PE matmuls for the per-batch tanh bias; depends only on
                the small DMAs so it runs while the big enc loads land."""
                for hc in range(HC):
                    p_ph = psm_pool.tile([128, B_LOC], f32, tag="sm")
                    for k in range(KH):
                        nc.tensor.matmul(
                            p_ph[:],
                            w_h[:, (k * HC + hc) * 128:(k * HC + hc + 1) * 128],
                            hidT[:, k * B_LOC:(k + 1) * B_LOC],
                            start=(k == 0), stop=False,
                        )
                    nc.tensor.matmul(
                        p_ph[:], battn_row[:, hc * 128:(hc + 1) * 128],
                        ones4[:], start=False, stop=True,
                    )
                    nc.any.tensor_copy(hbT[:, hc * B_LOC:(hc + 1) * B_LOC],
                                       p_ph[:])

            # ---------------- deferred emission machinery -----------------
            def emit_scores(sc):
                """Score-row matmuls for a finished batch: wv chunk stationary,
                energyT moving.  Accumulation groups (over hc) are contiguous
                per PSUM bank (start=True wipes the whole bank)."""
                b, enTs, p_srA, p_srB = sc
                for hc in range(HC):
                    nc.tensor.matmul(
                        p_srA[:], wv_col[:, hc:hc + 1], enTs[hc][:, 0:SA],
                        start=(hc == 0), stop=(hc == HC - 1),
                    )
                if SB:
                    for hc in range(HC):
                        nc.tensor.matmul(
                            p_srB[:], wv_col[:, hc:hc + 1],
                            enTs[hc][:, SA:s_p],
                            start=(hc == 0), stop=(hc == HC - 1),
                        )

            def emit_tail_a(pend):
                """Row softmax for a finished batch (DVE/ACT latency chain)."""
                pb, encT_b, p_srA, p_srB, t = pend
                sm = bpool.tile([1, s_p], f32, tag="sm")
                nc.vector.tensor_add(
                    sm[:, 0:SA], p_srA[:],
                    pb_sb[:, pb * s_p:pb * s_p + SA])
                if SB:
                    nc.vector.tensor_add(
                        sm[:, SA:s_p], p_srB[:],
                        pb_sb[:, pb * s_p + SA:(pb + 1) * s_p])
                p_exp = bpool.tile([1, s_p], f32, tag="p_exp")
                den = bpool.tile([1, 1], f32, tag="den")
                nc.scalar.activation(p_exp[:], sm[:], act.Exp,
                                     accum_out=den[:])
                rd = bpool.tile([1, 1], f32, tag="rd")
                nc.vector.reciprocal(rd[:], den[:])
                attn_row = bpool.tile([1, s_p], f32, tag="attn_row")
                nc.vector.tensor_scalar_mul(attn_row[:], p_exp[:], rd[:])
                nc.sync.dma_start(attn_d[pb][None, :], attn_row[:])
                attn_bf = bpool.tile([1, s_p], bf16, tag="attn_bf")
                nc.vector.tensor_scalar_mul(attn_bf[:], p_exp[:], rd[:])
                t.append(attn_bf)

            def emit_tail_b(pend):
                """Context for a finished batch: PE broadcast of the attention
                row, then fused multiply-reduce per e-chunk on the DVE."""
                pb, encT_b, p_srA, p_srB, t = pend
                attn_bf = t[0]
                bcA = psA_pool.tile([128, SA], f32, tag="pA")
                nc.tensor.matmul(bcA[:], ones1r[:], attn_bf[:, 0:SA],
                                 start=True, stop=True)
                if SB:
                    bcB = psB_pool.tile([128, SB], f32, tag="pB")
                    nc.tensor.matmul(bcB[:], ones1r[:], attn_bf[:, SA:s_p],
                                     start=True, stop=True)
                abf = bpool.tile([128, s_p], bf16, tag="abf")
                nc.vector.tensor_copy(abf[:, 0:SA], bcA[:])
                if SB:
                    nc.vector.tensor_copy(abf[:, SA:s_p], bcB[:])
                # (tensor_tensor_reduce hard-crashes this container's
                # runtime, so use two-op mult + reduce; reductions alternate
                # between DVE and the scalar engine's Copy-accumulate so no
                # single engine carries the whole contraction)
                ctx_sb = bpool.tile([128, KE], f32, tag="ctx_sb")
                for ec in range(KE):
                    scratch = bpool.tile([128, s_p], bf16, tag="scratch")
                    nc.vector.tensor_mul(
                        scratch[:], encT_b[:, ec * s_p:(ec + 1) * s_p], abf[:])
                    if ec % 2 == 0:
                        nc.vector.reduce_sum(ctx_sb[:, ec:ec + 1], scratch[:],
                                             axis=mybir.AxisListType.X)
                    else:
                        scr2 = bpool.tile([128, s_p], bf16, tag="scr2")
                        nc.scalar.activation(scr2[:], scratch[:], act.Copy,
                                             accum_out=ctx_sb[:, ec:ec + 1])
                nc.gpsimd.dma_start(ctx_d[pb], ctx_sb[:])

            # ---------------- main loop -----------------------------------
            emit_preamble()
            pend_sc = None    # scores awaiting emission
            pend_tail = None  # finished batch awaiting softmax+ctx
            encT_b = encT_first
            encT_next = None
            for it in range(n_iters):
                for b in range(B_LOC):
                    if not (it == 0 and b == 0):
                        encT_b = encT_next
                    last_batch = (it == n_iters - 1 and b == B_LOC - 1)
                    if not last_batch:
                        nb = (b + 1) % B_LOC
                        encT_next = encTpool.tile([128, KE * s_p], bf16,
                                                  tag="encT")
                        nc.scalar.dma_start(encT_next[:], enctp_d[nb])

                    p_srA = psr_pool.tile([1, SA], f32, tag="sr")
                    if SB:
                        p_srB = psr_pool.tile([1, SB], f32, tag="sr")
                    else:
                        p_srB = None
                    enTs = []
                    for hc in range(HC):
                        pA = psA_pool.tile([128, SA], f32, tag="pA")
                        if SB:
                            pB = psB_pool.tile([128, SB], f32, tag="pB")
                        for k in range(KE):
                            lhs = w_e[:, (k * HC + hc) * 128:
                                      (k * HC + hc + 1) * 128]
                            nc.tensor.matmul(
                                pA[:], lhs,
                                encT_b[:, k * s_p:k * s_p + SA],
                                start=(k == 0), stop=(k == KE - 1),
                            )
                            if SB:
                                nc.tensor.matmul(
                                    pB[:], lhs,
                                    encT_b[:, k * s_p + SA:(k + 1) * s_p],
                                    start=(k == 0), stop=(k == KE - 1),
                                )
                        if hc == 1 and pend_sc is not None:
                            emit_scores(pend_sc)
                            pend_sc = None
                        if hc == 2 and pend_tail is not None:
                            emit_tail_a(pend_tail)
                        if hc == 3 and pend_tail is not None:
                            emit_tail_b(pend_tail)
                            pend_tail = None
                        enT = enpool.tile([128, s_p], bf16, tag="enT")
                        hb_col = hbT[:, hc * B_LOC + b:hc * B_LOC + b + 1]
                        nc.scalar.activation(enT[:, 0:SA], pA[:], act.Tanh,
                                             bias=hb_col)
                        if SB:
                            nc.scalar.activation(enT[:, SA:s_p], pB[:],
                                                 act.Tanh, bias=hb_col)
                        enTs.append(enT)
                    pend_sc = (b, enTs, p_srA, p_srB)
                    if pend_tail is not None:
                        emit_tail_a(pend_tail)
                        emit_tail_b(pend_tail)
                    pend_tail = (b, encT_b, p_srA, p_srB, [])
                # end b loop
            if pend_sc is not None:
                emit_scores(pend_sc)
                pend_sc = None
            if pend_tail is not None:
                emit_tail_a(pend_tail)
                emit_tail_b(pend_tail)
                pend_tail = None

    _split_multiwaits(nc)
    import concourse.mybir as mybir2
    mybir2.codegen_inst_isa_subclasses(nc)
    return nc


def _get_nc(n_iters: int = 1, s_p: int = 552):
    key = ("nc", n_iters, s_p)
    if key not in _cache:
        _cache[key] = build_kernel(n_iters, s_p)
    return _cache[key]


def pick_s_p(mask):
    counts = mask.reshape(B, S).sum(1)
    m = int(counts.max())
    return max(128, min(S, ((m + 7) // 8) * 8))


def _pack_core(enc_c, mask_c, s_p):
    """Host-side pack: mask-selected rows, e-major bf16 tile layout + pad
    bias rows + scatter info."""
    enctp = np.zeros((B_LOC, 128, KE * s_p), bfloat16)
    pbias = np.zeros((B_LOC, s_p), np.float32)
    scat = []
    for b in range(B_LOC):
        idx = np.nonzero(mask_c[b])[0]
        n = len(idx)
        rows = enc_c[b, idx].astype(bfloat16)          # [n, E]
        t = np.zeros((s_p, E), bfloat16)
        t[:n] = rows
        enctp[b] = t.T.reshape(KE, 128, s_p).transpose(1, 0, 2).reshape(
            128, KE * s_p)
        pbias[b, n:] = NEG
        scat.append((n, idx.astype(np.int64)))
    return enctp, pbias.reshape(1, -1), scat


def shard_inputs(hidden, encoder_outputs, mask, W_attn, b_attn, W_v,
                 s_p=None):
    hidden = np.asarray(hidden, dtype=np.float32)
    enc = np.asarray(encoder_outputs, dtype=np.float32)
    mask = np.asarray(mask, dtype=np.int32)
    W_attn = np.asarray(W_attn, dtype=np.float32)
    b_attn = np.asarray(b_attn, dtype=np.float32)
    W_v = np.asarray(W_v, dtype=np.float32)
    if s_p is None:
        s_p = pick_s_p(mask)

    w_h = W_attn[:H].astype(bfloat16)                  # [512, 512]
    w_e = W_attn[H:].astype(bfloat16)                  # [1024, 512]
    wh_sb = np.ascontiguousarray(
        w_h.reshape(KH, 128, HC, 128).transpose(1, 0, 2, 3).reshape(
            128, KH * HC * 128))
    we_sb = np.ascontiguousarray(
        w_e.reshape(KE, 128, HC, 128).transpose(1, 0, 2, 3).reshape(
            128, KE * HC * 128))
    wv_col = np.ascontiguousarray(
        W_v.astype(bfloat16).reshape(HC, 128).T)       # [128, HC]
    battn_row = np.ascontiguousarray(
        b_attn.astype(bfloat16)[None, :])              # [1, H]

    in_maps, scats = [], []
    for c in range(N_CORES):
        sl = slice(c * B_LOC, (c + 1) * B_LOC)
        hidT = np.ascontiguousarray(
            hidden[sl].astype(bfloat16).T.reshape(
                KH, 128, B_LOC).transpose(1, 0, 2).reshape(128, KH * B_LOC))
        enctp, pb_rows, scat = _pack_core(enc[sl], mask[sl], s_p)
        in_maps.append({
            "enctp": enctp,
            "wh_sb": wh_sb,
            "we_sb": we_sb,
            "hidT": hidT,
            "wv_col": wv_col,
            "battn_row": battn_row,
            "pb_rows": pb_rows,
        })
        scats.append(scat)
    return in_maps, scats, s_p


def kernel(hidden, encoder_outputs, mask, W_attn, b_attn, W_v):
    from concourse.bass_utils import run_bass_kernel_spmd

    in_maps, scats, s_p = shard_inputs(
        hidden, encoder_outputs, mask, W_attn, b_attn, W_v)
    nc = _get_nc(1, s_p)
    res = run_bass_kernel_spmd(nc, in_maps, list(range(N_CORES)))
    context = np.zeros((B, E), np.float32)
    attn_w = np.zeros((B, S), np.float32)
    for c in range(N_CORES):
        attn_p = res.results[c]["out_attn"]          # [B_LOC, s_p]
        ctx_c = res.results[c]["out_ctxcols"]        # [B_LOC, 128, KE]
        for b in range(B_LOC):
            n, idx = scats[c][b]
            attn_w[c * B_LOC + b, idx] = attn_p[b, :n]
            context[c * B_LOC + b] = ctx_c[b].T.reshape(-1)  # e = ec*128+p
    return context, attn_w
